# revision 1
# baseline (speedup 1.0000x reference)
"""Trainium2 Bass kernel for the 3-layer GAT denoising model
(nn_Denoising_Model_24764781429262): N=50000 nodes, E=800000 edges, 8 heads.

Strategy (8 NeuronCores):
- Host: add self-loops, assign each node to half A/B (balanced by degree),
  sort each half by (in-degree-from-A, in-degree-from-B), and pack nodes into
  128-node destination tiles with near-uniform padded ELL width. Tiles are
  dealt round-robin to the 8 cores; each core owns 49 contiguous table tiles.
- Per layer: each core computes its shard of a per-node table
  [H(256) | alpha_src(8) | alpha_dst(8)] with PE matmuls (fused into the
  previous layer's epilogue), AllGather's the table, then runs the edge phase:
  dma_gather of padded per-slot source rows (A/B half-tables keep int16
  indices in range), w = exp(leakyrelu(alpha_s + alpha_d)), weighted
  sum over slots on DVE -> num/denom in SBUF, divide + temb + bias + ELU.
- Final MLP is node-sharded; host inverse-permutes the output.

kernel(**inputs) takes the full unsharded inputs and returns the full
[50000, 8] float32 output.
"""

import math
import os
import numpy as np

os.environ.setdefault("NEURON_RT_RESET_CORES", "1")

import concourse.bacc as bacc
import concourse.mybir as mybir
import concourse.tile as tile
from concourse.masks import make_identity

N_CORES = 8
C = 320
HC = 256
NH = 8
F32 = mybir.dt.float32
I16 = mybir.dt.int16
AF = mybir.ActivationFunctionType
OP = mybir.AluOpType


# ----------------------------------------------------------------------------
# host preprocessing
# ----------------------------------------------------------------------------
def preprocess(adj, n, NT):
    L = 128 * NT
    PB = L + 8
    NSLOT = N_CORES * L
    HALF = 4 * PB
    E = adj.shape[1]
    src = np.concatenate([adj[0], np.arange(n)]).astype(np.int64)
    dst = np.concatenate([adj[1], np.arange(n)]).astype(np.int64)

    deg = np.bincount(dst, minlength=n)
    order_tot = np.argsort(deg, kind="stable")
    half_bit = np.zeros(n, dtype=bool)
    half_bit[order_tot[1::2]] = True
    src_is_b = half_bit[src]
    degA = np.bincount(dst[~src_is_b], minlength=n)
    degB = np.bincount(dst[src_is_b], minlength=n)

    A_nodes = np.flatnonzero(~half_bit)
    B_nodes = np.flatnonzero(half_bit)
    A_sorted = A_nodes[np.lexsort((degB[A_nodes], degA[A_nodes]))]
    B_sorted = B_nodes[np.lexsort((degB[B_nodes], degA[B_nodes]))]
    HS = NSLOT // 2
    assert len(A_sorted) <= HS and len(B_sorted) <= HS
    A_list = np.concatenate([np.full(HS - len(A_sorted), -1, np.int64), A_sorted])
    B_list = np.concatenate([np.full(HS - len(B_sorted), -1, np.int64), B_sorted])

    slots = np.full(NSLOT, -1, dtype=np.int64)
    r = np.arange(NSLOT)
    t = r // 128
    k = t % N_CORES
    i = t // N_CORES
    p = r % 128
    jA = i * 4 + k
    jB = i * 4 + (k - 4)
    selA = k < 4
    slots[selA] = A_list[jA[selA] * 128 + p[selA]]
    slots[~selA] = B_list[jB[~selA] * 128 + p[~selA]]
    physrow = k * PB + i * 128 + p
    node2phys = np.full(n, -1, dtype=np.int64)
    real = slots >= 0
    node2phys[slots[real]] = physrow[real]
    assert (node2phys >= 0).all()
    assert (node2phys[A_nodes] < HALF).all()
    assert (node2phys[B_nodes] >= HALF).all()

    dphys = node2phys[dst]
    dk = dphys // PB
    dloc = dphys % PB
    di = dloc // 128
    dp = dloc % 128

    a_cnt = np.zeros(n, np.int64)
    b_cnt = np.zeros(n, np.int64)
    np.add.at(a_cnt, dst[~src_is_b], 1)
    np.add.at(b_cnt, dst[src_is_b], 1)

    DA = np.zeros((N_CORES, NT), np.int64)
    DB = np.zeros((N_CORES, NT), np.int64)
    node_k = node2phys // PB
    node_i = (node2phys % PB) // 128
    np.maximum.at(DA, (node_k, node_i), a_cnt)
    np.maximum.at(DB, (node_k, node_i), b_cnt)
    DAi = np.maximum(DA.max(axis=0), 1)
    DBi = np.maximum(DB.max(axis=0), 1)

    # idx blocks per (core, tile, half); dummy local idx = L
    coreA = [[np.full(128 * DAi[ii], L, np.int32) for ii in range(NT)]
             for _ in range(N_CORES)]
    coreB = [[np.full(128 * DBi[ii], L, np.int32) for ii in range(NT)]
             for _ in range(N_CORES)]

    # per-(dst, half) cumulative rank
    es = np.lexsort((src, dst))
    ds_, isb_ = dst[es], src_is_b[es]
    dk_, di_, dp_ = dk[es], di[es], dp[es]
    sphys_ = node2phys[src[es]]
    keys = ds_ * 2 + isb_.astype(np.int64)
    sort2 = np.argsort(keys, kind="stable")
    ks = keys[sort2]
    starts = np.r_[0, np.flatnonzero(np.diff(ks)) + 1]
    cum = np.arange(len(ks))
    seg_start = np.repeat(cum[starts], np.diff(np.r_[starts, len(ks)]))
    rank = cum - seg_start
    jcol = np.empty(len(ks), np.int64)
    jcol[sort2] = rank
    # vectorized scatter into the per-(core,tile) blocks
    flatA_off = np.zeros((N_CORES, NT), np.int64)
    flatB_off = np.zeros((N_CORES, NT), np.int64)
    sizesA = 128 * DAi
    sizesB = 128 * DBi
    offA = np.concatenate([[0], np.cumsum(sizesA)[:-1]])
    offB = np.concatenate([[0], np.cumsum(sizesB)[:-1]])
    bigA = [np.concatenate(coreA[kk]) for kk in range(N_CORES)]
    bigB = [np.concatenate(coreB[kk]) for kk in range(N_CORES)]
    selB = isb_
    posA = offA[di_[~selB]] + jcol[~selB] * 128 + dp_[~selB]
    posB = offB[di_[selB]] + jcol[selB] * 128 + dp_[selB]
    for kk in range(N_CORES):
        mA = (~selB) & (dk_ == kk)
        bigA[kk][offA[di_[mA]] + jcol[mA] * 128 + dp_[mA]] = sphys_[mA]
        mB = selB & (dk_ == kk)
        bigB[kk][offB[di_[mB]] + jcol[mB] * 128 + dp_[mB]] = sphys_[mB] - HALF
    for kk in range(N_CORES):
        for ii in range(NT):
            coreA[kk][ii] = bigA[kk][offA[ii]:offA[ii] + sizesA[ii]]
            coreB[kk][ii] = bigB[kk][offB[ii]:offB[ii] + sizesB[ii]]

    return dict(slots=slots, node2phys=node2phys, DAi=DAi, DBi=DBi,
                coreA=coreA, coreB=coreB, n=n, NT=NT, L=L, PB=PB,
                NSLOT=NSLOT, HALF=HALF)


def build_chunks(prep, cmax):
    """Chunk plan shared by all cores: list of (tile_i, half, col0, ncols)."""
    chunks = []
    for ii in range(prep["NT"]):
        for half, D in (("A", prep["DAi"][ii]), ("B", prep["DBi"][ii])):
            c0 = 0
            while c0 < D:
                cc = min(cmax, D - c0)
                chunks.append((ii, half, c0, int(cc)))
                c0 += cc
    return chunks


def wrap_idx(block_i32):
    num = block_i32.shape[0]
    assert num % 16 == 0
    g = block_i32.reshape(num // 16, 16).T.astype(np.int16)
    return np.tile(g, (8, 1))  # [128, num/16]


def host_inputs(inputs, prep, chunks):
    """Build per-core input maps (numpy) for the bass program."""
    n, NT, L, PB = prep["n"], prep["NT"], prep["L"], prep["PB"]
    x = np.asarray(inputs["x"], np.float32)
    qY = np.asarray(inputs["q_Y_sample"], np.float32)
    NF = x.shape[1]
    F0 = NF + qY.shape[1]

    slots = prep["slots"]
    r_real = np.flatnonzero(slots >= 0)
    nodes = slots[r_real]

    # slot-order full arrays
    NS = prep["NSLOT"]
    h0 = np.zeros((NS, F0), np.float32)
    qYs = np.zeros((NS, NH), np.float32)
    h0[r_real, :NF] = x[nodes]
    h0[r_real, NF:] = qY[nodes]
    qYs[r_real] = qY[nodes]

    # per-core shard slices in slot space: core k's tile i = slot-tile t=i*8+k
    # slot index of (k, i, p) = (i*8+k)*128 + p
    def shard_rows(k):
        idx = np.empty(L, np.int64)
        for i in range(NT):
            idx[i * 128:(i + 1) * 128] = (i * N_CORES + k) * 128 + np.arange(128)
        return idx

    # weights
    W = [np.asarray(inputs[f"W{i}"], np.float32) for i in range(3)]
    att_src = np.asarray(inputs["att_src"], np.float32)
    att_dst = np.asarray(inputs["att_dst"], np.float32)
    bias = np.asarray(inputs["bias"], np.float32)
    Whats = []
    for l in range(3):
        As = np.zeros((HC, NH), np.float32)
        Ad = np.zeros((HC, NH), np.float32)
        for hh in range(NH):
            As[hh * 32:(hh + 1) * 32, hh] = att_src[l, hh]
            Ad[hh * 32:(hh + 1) * 32, hh] = att_dst[l, hh]
        Wh = np.zeros((W[l].shape[0], C), np.float32)
        Wh[:, :HC] = W[l]
        Wh[:, HC:HC + NH] = W[l] @ As
        Wh[:, HC + NH:HC + 2 * NH] = W[l] @ Ad
        Whats.append(Wh)
    # pad What0 to 136 rows already is; What1/2 264 rows.

    half = 64
    freqs4 = np.exp(np.arange(half, dtype=np.float32)
                    * (-math.log(10000.0) / (half - 1))).astype(np.float32)
    b_rep = np.stack([np.tile(bias[l][None, :], (128, 1)) for l in range(3)])

    fin_w1 = np.asarray(inputs["fin_w1"], np.float32)
    fin_b1 = np.asarray(inputs["fin_b1"], np.float32)
    fin_w2 = np.asarray(inputs["fin_w2"], np.float32)
    fin_b2 = np.asarray(inputs["fin_b2"], np.float32)

    dummy = np.zeros((8, C), np.float32)
    dummy[:, HC:HC + NH] = -1e4

    common = {
        "What0": Whats[0], "What1": Whats[1], "What2": Whats[2],
        "b_rep": b_rep.astype(np.float32),
        "fin_w1": fin_w1, "fin_b1rep": np.tile(fin_b1[None, :], (128, 1)).astype(np.float32),
        "fin_w2": fin_w2, "fin_b2rep": np.tile(fin_b2[None, :], (128, 1)).astype(np.float32),
        "tmlp_w1": np.asarray(inputs["tmlp_w1"], np.float32),
        "tmlp_b1col": np.asarray(inputs["tmlp_b1"], np.float32).reshape(128, 1),
        "tmlp_w2": np.asarray(inputs["tmlp_w2"], np.float32),
        "tmlp_b2col": np.asarray(inputs["tmlp_b2"], np.float32).reshape(256, 1)[:, :],
        "freqs4": freqs4.reshape(half, 1),
        "t_in": np.asarray(inputs["t"], np.float32).reshape(1, 1),
        "dummy_in": dummy,
    }
    # tmlp_b2col is [256,1]; split into [128,2] column pair for psum adds
    b2c = common.pop("tmlp_b2col")
    common["tmlp_b2cols"] = np.concatenate([b2c[:128], b2c[128:]], axis=1)  # [128,2]

    in_maps = []
    for k in range(N_CORES):
        rows = shard_rows(k)
        idx_blocks = []
        for (ii, hf, c0, cc) in chunks:
            blk = (prep["coreA"][k][ii] if hf == "A" else prep["coreB"][k][ii])
            sub = blk[c0 * 128:(c0 + cc) * 128]
            idx_blocks.append(wrap_idx(sub))
        idx_all = np.concatenate(idx_blocks, axis=1)  # [128, total/16]
        m = dict(common)
        m["h0T_shard"] = np.ascontiguousarray(h0[rows].T)          # [F0, L]
        m["qY_shard"] = np.ascontiguousarray(qYs[rows])            # [L, 8]
        m["idx_all"] = np.ascontiguousarray(idx_all)
        in_maps.append(m)
    return in_maps


# ----------------------------------------------------------------------------
# bass program
# ----------------------------------------------------------------------------
def build_program(prep, chunks, cmax, F0=136):
    NT, L, PB, HALF = prep["NT"], prep["L"], prep["PB"], prep["HALF"]
    NROWS = N_CORES * PB
    IDXC = sum(cc * 8 for (_, _, _, cc) in chunks)

    nc = bacc.Bacc("TRN2", target_bir_lowering=False, debug=False,
                   enable_asserts=False, num_devices=N_CORES)

    # inputs
    h0T = nc.dram_tensor("h0T_shard", [F0, L], F32, kind="ExternalInput")
    qYs = nc.dram_tensor("qY_shard", [L, NH], F32, kind="ExternalInput")
    idx_all = nc.dram_tensor("idx_all", [128, IDXC], I16, kind="ExternalInput")
    What = [nc.dram_tensor(f"What{l}", [F0 if l == 0 else 264, C], F32,
                           kind="ExternalInput") for l in range(3)]
    b_rep = nc.dram_tensor("b_rep", [3, 128, HC], F32, kind="ExternalInput")
    fin_w1 = nc.dram_tensor("fin_w1", [264, 528], F32, kind="ExternalInput")
    fin_b1rep = nc.dram_tensor("fin_b1rep", [128, 528], F32, kind="ExternalInput")
    fin_w2 = nc.dram_tensor("fin_w2", [528, NH], F32, kind="ExternalInput")
    fin_b2rep = nc.dram_tensor("fin_b2rep", [128, NH], F32, kind="ExternalInput")
    tw1 = nc.dram_tensor("tmlp_w1", [128, 128], F32, kind="ExternalInput")
    tb1c = nc.dram_tensor("tmlp_b1col", [128, 1], F32, kind="ExternalInput")
    tw2 = nc.dram_tensor("tmlp_w2", [128, HC], F32, kind="ExternalInput")
    tb2c = nc.dram_tensor("tmlp_b2cols", [128, 2], F32, kind="ExternalInput")
    freqs4 = nc.dram_tensor("freqs4", [64, 1], F32, kind="ExternalInput")
    t_in = nc.dram_tensor("t_in", [1, 1], F32, kind="ExternalInput")
    dummy_in = nc.dram_tensor("dummy_in", [8, C], F32, kind="ExternalInput")

    out = nc.dram_tensor("out", [L, NH], F32, kind="ExternalOutput")

    # internals
    AGIN = [nc.dram_tensor(f"agin{l}", [PB, C], F32, kind="Internal")
            for l in range(3)]
    T = [nc.dram_tensor(f"table{l}", [NROWS, C], F32, kind="Internal",
                        addr_space="Shared") for l in range(3)]

    with tile.TileContext(nc) as tc:
        import contextlib
        with contextlib.ExitStack() as ctx:
            consts = ctx.enter_context(tc.tile_pool(name="consts", bufs=1))
            sb = ctx.enter_context(tc.tile_pool(name="sb", bufs=3))
            sb3 = ctx.enter_context(tc.tile_pool(name="sb3", bufs=3))
            ps = ctx.enter_context(tc.tile_pool(name="ps", bufs=2, space="PSUM"))
            ps1 = ctx.enter_context(tc.tile_pool(name="ps1", bufs=1, space="PSUM"))
            gp = ctx.enter_context(tc.tile_pool(name="gp", bufs=3))

            ident = consts.tile([128, 128], F32)
            make_identity(nc, ident[:])

            # ---- dummy rows into AGIN tails
            for l in range(3):
                dt_ = consts.tile([8, C], F32, tag="dummyt")
                nc.sync.dma_start(out=dt_[:], in_=dummy_in[:])
                nc.sync.dma_start(out=AGIN[l][L:PB, :], in_=dt_[:])

            # ---- temb -> tb[l] tiles [128, 256]
            tcol = consts.tile([64, 1], F32, tag="tcol")
            nc.sync.dma_start(out=tcol[0:1, :], in_=t_in[:])
            nc.gpsimd.partition_broadcast(out_ap=tcol[:], in_ap=tcol[0:1, :])
            fq = consts.tile([64, 1], F32, tag="fq")
            nc.sync.dma_start(out=fq[:], in_=freqs4[:])
            # xs = t * 4 (t/num_steps*num_steps*rescale cancels; *4 is exact)
            xs = consts.tile([64, 1], F32, tag="xs")
            nc.vector.tensor_scalar_mul(xs[:], tcol[:], 4.0)
            ang = consts.tile([64, 1], F32, tag="ang")
            nc.vector.tensor_tensor(out=ang[:], in0=xs[:], in1=fq[:], op=OP.mult)
            # range-reduce ang into [-pi, pi]: k = floor(ang/2pi + .5), Cody-Waite
            TWO_PI = 2 * math.pi
            c1 = float(np.float32(TWO_PI))
            c2 = float(np.float32(TWO_PI - c1))
            c3 = float(TWO_PI - c1 - float(np.float32(TWO_PI - c1)))
            yk = consts.tile([64, 1], F32, tag="yk")
            nc.vector.tensor_scalar_mul(yk[:], ang[:], 1.0 / TWO_PI)
            ki = consts.tile([64, 1], mybir.dt.int32, tag="ki")
            nc.vector.tensor_copy(out=ki[:], in_=yk[:])
            kk_t = consts.tile([64, 1], F32, tag="kk_t")
            nc.vector.tensor_copy(out=kk_t[:], in_=ki[:])
            red = consts.tile([64, 1], F32, tag="red")
            nc.vector.cody_waite_cascade(out=red[:], x=ang[:], k=kk_t[:],
                                         c1=c1, c2=c2, c3=c3)
            rs = consts.tile([64, 1], F32, tag="rs")
            rc = consts.tile([64, 1], F32, tag="rc")
            nc.vector.add_range_wrap(out=rs[:], in_=red[:], shift=0.0,
                                     bound=math.pi, period=TWO_PI)
            nc.vector.add_range_wrap(out=rc[:], in_=red[:], shift=math.pi / 2,
                                     bound=math.pi, period=TWO_PI)
            sc = consts.tile([128, 1], F32, tag="sc")
            sc2 = consts.tile([64, 1], F32, tag="sc2")
            nc.scalar.activation(sc[0:64, :], rs[:], AF.Sin)
            nc.scalar.activation(sc2[:], rc[:], AF.Sin)
            nc.sync.dma_start(out=sc[64:128, :], in_=sc2[:])

            def elu_(xap, tmp_pool, shape, tag):
                # in-place ELU on xap: x = relu(x) + min(exp(x)-1, 0)
                e = tmp_pool.tile(shape, F32, tag=tag + "_e")
                r = tmp_pool.tile(shape, F32, tag=tag + "_r")
                nc.scalar.activation(e[:], xap, AF.Exp)
                nc.vector.tensor_scalar(out=e[:], in0=e[:], scalar1=-1.0,
                                        scalar2=0.0, op0=OP.add, op1=OP.min)
                nc.scalar.activation(r[:], xap, AF.Relu)
                nc.vector.tensor_tensor(out=xap, in0=e[:], in1=r[:], op=OP.add)

            tw1_s = consts.tile([128, 128], F32, tag="tw1")
            nc.sync.dma_start(out=tw1_s[:], in_=tw1[:])
            tw2_s = consts.tile([128, HC], F32, tag="tw2")
            nc.sync.dma_start(out=tw2_s[:], in_=tw2[:])
            e1p = ps1.tile([128, 1], F32, tag="tembp")
            nc.tensor.matmul(out=e1p[:], lhsT=tw1_s[:], rhs=sc[:], start=True, stop=True)
            b1c = consts.tile([128, 1], F32, tag="tb1c")
            nc.sync.dma_start(out=b1c[:], in_=tb1c[:])
            e1 = consts.tile([128, 1], F32, tag="e1")
            nc.vector.tensor_tensor(out=e1[:], in0=e1p[:], in1=b1c[:], op=OP.add)
            elu_(e1[:], consts, [128, 1], "elu_temb")
            tcols_p = ps1.tile([128, 2], F32, tag="tembp")
            nc.tensor.matmul(out=tcols_p[:, 0:1], lhsT=tw2_s[:, 0:128], rhs=e1[:],
                             start=True, stop=True)
            nc.tensor.matmul(out=tcols_p[:, 1:2], lhsT=tw2_s[:, 128:256], rhs=e1[:],
                             start=True, stop=True)
            b2c = consts.tile([128, 2], F32, tag="tb2c")
            nc.sync.dma_start(out=b2c[:], in_=tb2c[:])
            tcols = consts.tile([128, 2], F32, tag="tcols")
            nc.vector.tensor_tensor(out=tcols[:], in0=tcols_p[:], in1=b2c[:], op=OP.add)
            trow_p = ps1.tile([2, 128], F32, tag="tembp")
            nc.tensor.transpose(out=trow_p[:], in_=tcols[:], identity=ident[:])
            trow2 = consts.tile([2, 128], F32, tag="trow2")
            nc.scalar.copy(out=trow2[:], in_=trow_p[:])
            trow = consts.tile([1, HC], F32, tag="trow")
            nc.sync.dma_start(out=trow[0:1, 0:128], in_=trow2[0:1, :])
            nc.sync.dma_start(out=trow[0:1, 128:256], in_=trow2[1:2, :])
            temb_rep = consts.tile([128, HC], F32, tag="temb_rep")
            nc.gpsimd.partition_broadcast(out_ap=temb_rep[:], in_ap=trow[:])
            tb = []
            for l in range(3):
                bl = consts.tile([128, HC], F32, tag=f"b_rep{l}")
                nc.sync.dma_start(out=bl[:], in_=b_rep[l])
                tbl = consts.tile([128, HC], F32, tag=f"tb{l}")
                nc.vector.tensor_tensor(out=tbl[:], in0=temb_rep[:], in1=bl[:], op=OP.add)
                tb.append(tbl)

            # ---- layer-l What chunk tiles (load all 3 layers up-front; small)
            Wchunks = []
            for l in range(3):
                F = F0 if l == 0 else 264
                cks = []
                off = 0
                while off < F:
                    kk = min(128, F - off)
                    wt = consts.tile([kk, C], F32, tag=f"W{l}_{off}")
                    nc.sync.dma_start(out=wt[:], in_=What[l][off:off + kk, :])
                    cks.append((wt, kk))
                    off += kk
                Wchunks.append(cks)
            fw1 = []
            off = 0
            while off < 264:
                kk = min(128, 264 - off)
                wt = consts.tile([kk, 528], F32, tag=f"fw1_{off}")
                nc.sync.dma_start(out=wt[:], in_=fin_w1[off:off + kk, :])
                fw1.append((wt, kk))
                off += kk
            fw2 = []
            off = 0
            while off < 528:
                kk = min(128, 528 - off)
                wt = consts.tile([kk, NH], F32, tag=f"fw2_{off}")
                nc.sync.dma_start(out=wt[:], in_=fin_w2[off:off + kk, :])
                fw2.append((wt, kk))
                off += kk
            fb1 = consts.tile([128, 528], F32, tag="fb1")
            nc.sync.dma_start(out=fb1[:], in_=fin_b1rep[:])
            fb2 = consts.tile([128, NH], F32, tag="fb2")
            nc.sync.dma_start(out=fb2[:], in_=fin_b2rep[:])

            # ---- helper: dense T-row compute from hT chunks
            def dense_tile(hT_chunks, l, i, agin):
                pT = ps.tile([128, C], F32, tag="pT")
                ncks = len(Wchunks[l])
                for ci, ((wt, kk), (ht, kk2)) in enumerate(zip(Wchunks[l], hT_chunks)):
                    assert kk == kk2, (kk, kk2)
                    nc.tensor.matmul(out=pT[:], lhsT=ht[:kk, :], rhs=wt[:],
                                     start=(ci == 0), stop=(ci == ncks - 1))
                Trow_s = sb.tile([128, C], F32, tag="Trow_s")
                nc.scalar.copy(out=Trow_s[:], in_=pT[:])
                nc.sync.dma_start(out=agin[i * 128:(i + 1) * 128, :], in_=Trow_s[:])

            # ---- layer 0 dense: from h0T input
            for i in range(NT):
                hts = []
                off = 0
                while off < F0:
                    kk = min(128, F0 - off)
                    ht = sb.tile([128, 128], F32, tag=f"h0t_{off}")
                    nc.sync.dma_start(out=ht[:kk, 0:128],
                                      in_=h0T[off:off + kk, i * 128:(i + 1) * 128])
                    hts.append((ht, kk))
                    off += kk
                dense_tile(hts, 0, i, AGIN[0])

            # ---- per layer: allgather + edge phase
            idx_off = [0]  # running column offset in idx_all

            def edge_layer(l):
                """Edge phase for layer l; produces h_{l+1} tiles and either
                fused dense into AGIN[l+1] or the final MLP into out."""
                TA = T[l][0:HALF, :]
                TB = T[l][HALF:2 * HALF, :]
                # per-tile loop
                ch_by_tile = {}
                for ch in chunks:
                    ch_by_tile.setdefault(ch[0], []).append(ch)
                idxc = 0
                for i in range(NT):
                    acc_num = sb.tile([128, HC], F32, tag="acc_num")
                    acc_den = sb.tile([128, NH], F32, tag="acc_den")
                    first_chunk = True
                    ad_t = sb.tile([128, NH], F32, tag="ad_t")
                    row0 = 0 * PB + i * 128  # own shard: rows i*128.. of OWN block
                    # own shard rows in the big table: core k's block. SPMD: the
                    # program must address rows of ITS OWN shard. But the
                    # program is identical across cores! Use partition id?
                    # -> handled via own-shard AGIN copy: alpha_dst read from
                    #    AGIN[l] (own shard block) instead of T[l].
                    nc.sync.dma_start(
                        out=ad_t[:],
                        in_=AGIN[l][i * 128:(i + 1) * 128, HC + NH:HC + 2 * NH])
                    for (ii, hf, c0, cc) in ch_by_tile[i]:
                        tbl = TA if hf == "A" else TB
                        idx_t = sb3.tile([128, cmax * 8], I16, tag="idx_t")
                        nc.sync.dma_start(out=idx_t[:, 0:cc * 8],
                                          in_=idx_all[:, idxc:idxc + cc * 8])
                        g_t = gp.tile([128, cmax, C], F32, tag="g_t")
                        nc.gpsimd.dma_gather(
                            out_ap=g_t[:, 0:cc, :], in_ap=tbl,
                            idxs_ap=idx_t[:, 0:cc * 8],
                            num_idxs=128 * cc, num_idxs_reg=128 * cc,
                            elem_size=C, single_packet=False)
                        idxc += cc * 8
                        # logits = alpha_src + alpha_dst
                        lg = sb.tile([128, cmax, NH], F32, tag="lg")
                        nc.vector.tensor_tensor(
                            out=lg[:, 0:cc, :],
                            in0=g_t[:, 0:cc, HC:HC + NH],
                            in1=ad_t[:].unsqueeze(1).broadcast_to([128, cc, NH]),
                            op=OP.add)
                        # w = exp(leakyrelu_0.2)
                        nc.vector.scalar_tensor_tensor(
                            out=lg[:, 0:cc, :], in0=lg[:, 0:cc, :], scalar=0.2,
                            in1=lg[:, 0:cc, :], op0=OP.mult, op1=OP.max)
                        w_t = sb.tile([128, cmax, NH], F32, tag="w_t")
                        nc.scalar.activation(w_t[:, 0:cc, :], lg[:, 0:cc, :], AF.Exp)
                        # denom partial
                        dtar = acc_den if first_chunk else sb.tile([128, NH], F32, tag="dpart")
                        nc.vector.tensor_reduce(
                            out=dtar[:, :, None],
                            in_=w_t[:, 0:cc, :].rearrange("p j h -> p h j"),
                            axis=mybir.AxisListType.X, op=OP.add)
                        if not first_chunk:
                            nc.vector.tensor_tensor(out=acc_den[:], in0=acc_den[:],
                                                    in1=dtar[:], op=OP.add)
                        # num partial: tmp[p, c, j] = g[p, j, c] * w[p, j, h(c)]
                        tmp = sb.tile([128, HC, cmax], F32, tag="tmp")
                        gv = g_t[:, 0:cc, 0:HC].rearrange(
                            "p j (h c) -> p h c j", h=NH)
                        wv = w_t[:, 0:cc, :].rearrange("p j h -> p h j") \
                            .unsqueeze(2).broadcast_to([128, NH, 32, cc])
                        tv = tmp[:, :, 0:cc].rearrange("p (h c) j -> p h c j", h=NH)
                        nc.vector.tensor_tensor(out=tv, in0=gv, in1=wv, op=OP.mult)
                        ntar = acc_num if first_chunk else sb.tile([128, HC], F32, tag="npart")
                        nc.vector.tensor_reduce(
                            out=ntar[:, :, None], in_=tmp[:, :, 0:cc],
                            axis=mybir.AxisListType.X, op=OP.add)
                        if not first_chunk:
                            nc.vector.tensor_tensor(out=acc_num[:], in0=acc_num[:],
                                                    in1=ntar[:], op=OP.add)
                        first_chunk = False
                    # ---- finalize tile -> h_tile
                    rcp = sb.tile([128, NH], F32, tag="rcp")
                    nc.vector.reciprocal(rcp[:], acc_den[:])
                    h_t = sb.tile([128, 264], F32, tag="h_t")
                    nc.vector.tensor_tensor(
                        out=h_t[:, 0:HC].rearrange("p (h c) -> p h c", h=NH),
                        in0=acc_num[:].rearrange("p (h c) -> p h c", h=NH),
                        in1=rcp[:].unsqueeze(2).broadcast_to([128, NH, 32]),
                        op=OP.mult)
                    nc.vector.tensor_tensor(out=h_t[:, 0:HC], in0=h_t[:, 0:HC],
                                            in1=tb[l][:], op=OP.add)
                    elu_(h_t[:, 0:HC], sb, [128, HC], "elu_h")
                    nc.sync.dma_start(out=h_t[:, HC:264],
                                      in_=qYs[i * 128:(i + 1) * 128, :])
                    # ---- transposes
                    hts = []
                    for (off, kk, tg) in ((0, 128, "hT0"), (128, 128, "hT1"),
                                          (256, 8, "hT2")):
                        pt = ps.tile([kk, 128] if kk < 128 else [128, 128], F32,
                                     tag="ptr")
                        nc.tensor.transpose(out=pt[:kk, :],
                                            in_=h_t[:, off:off + kk],
                                            identity=ident[:])
                        st = sb.tile([kk, 128] if kk < 128 else [128, 128], F32,
                                     tag=tg)
                        nc.scalar.copy(out=st[:kk, :], in_=pt[:kk, :])
                        hts.append((st, kk))
                    if l < 2:
                        dense_tile(hts, l + 1, i, AGIN[l + 1])
                    else:
                        # final MLP
                        u = sb.tile([128, 528], F32, tag="u")
                        for half_i in range(2):
                            pm = ps1.tile([128, 264], F32, tag="pmlp")
                            for ci, (ht, kk) in enumerate(hts):
                                nc.tensor.matmul(
                                    out=pm[:],
                                    lhsT=ht[:kk, :],
                                    rhs=fw1[ci][0][:kk, half_i * 264:(half_i + 1) * 264],
                                    start=(ci == 0), stop=(ci == 2))
                            nc.vector.tensor_tensor(
                                out=u[:, half_i * 264:(half_i + 1) * 264],
                                in0=pm[:],
                                in1=fb1[:, half_i * 264:(half_i + 1) * 264],
                                op=OP.add)
                        elu_(u[:], sb, [128, 528], "elu_u")
                        po = ps1.tile([128, NH], F32, tag="po")
                        uTs = []
                        for ci in range(5):
                            off = ci * 128
                            kk = min(128, 528 - off)
                            pt = ps.tile([128, 128], F32, tag="ptr")
                            nc.tensor.transpose(out=pt[:kk, :],
                                                in_=u[:, off:off + kk],
                                                identity=ident[:])
                            st = sb.tile([128, 128], F32, tag=f"uT{ci}")
                            nc.scalar.copy(out=st[:kk, :], in_=pt[:kk, :])
                            uTs.append((st, kk))
                        for ci, (st, kk) in enumerate(uTs):
                            nc.tensor.matmul(out=po[:], lhsT=st[:kk, :],
                                             rhs=fw2[ci][0][:kk, :],
                                             start=(ci == 0), stop=(ci == 4))
                        o_t = sb.tile([128, NH], F32, tag="o_t")
                        nc.vector.tensor_tensor(out=o_t[:], in0=po[:], in1=fb2[:],
                                                op=OP.add)
                        nc.sync.dma_start(out=out[i * 128:(i + 1) * 128, :],
                                          in_=o_t[:])

            for l in range(3):
                nc.gpsimd.collective_compute(
                    "AllGather", OP.bypass,
                    replica_groups=[list(range(N_CORES))],
                    ins=[AGIN[l][:]], outs=[T[l][:]])
                edge_layer(l)

    nc.compile()
    return nc


def run(inputs, NT=49, cmax=8, trace=False):
    from concourse.bass_utils import run_bass_kernel_spmd
    from concourse.bass_interp import get_hw_module
    adj = np.asarray(inputs["adj"])
    n = int(np.asarray(inputs["x"]).shape[0])
    prep = preprocess(adj, n, NT)
    chunks = build_chunks(prep, cmax)
    in_maps = host_inputs(inputs, prep, chunks)
    F0 = in_maps[0]["h0T_shard"].shape[0]
    nc = build_program(prep, chunks, cmax, F0=F0)
    nc.m = get_hw_module(nc.m)
    res = run_bass_kernel_spmd(nc, in_maps, core_ids=list(range(N_CORES)),
                               trace=trace)
    outs = [np.asarray(r["out"]) for r in res.results]
    NS = prep["NSLOT"]
    y_slots = np.zeros((NS, NH), np.float32)
    L = prep["L"]
    for k in range(N_CORES):
        for i in range(NT):
            slot_base = (i * N_CORES + k) * 128
            y_slots[slot_base:slot_base + 128] = outs[k][i * 128:(i + 1) * 128]
    slots = prep["slots"]
    r_real = np.flatnonzero(slots >= 0)
    y = np.zeros((n, NH), np.float32)
    y[slots[r_real]] = y_slots[r_real]
    return y, res


def kernel(**inputs) -> np.ndarray:
    y, _ = run(inputs, NT=49, cmax=16)
    return y



# revision 3
# speedup vs baseline: 1.6500x; 1.6500x over previous
"""Trainium2 Bass kernel for the 3-layer GAT denoising model
(nn_Denoising_Model_24764781429262): N=50000 nodes, E=800000 edges, 8 heads.

Restructured design (vs. the f32 table-AllGather baseline):
- bf16 table [NROWS, 384]: H in c-major layout (cols 0:256), ones cols
  256:264 (softmax denominator folded into the main weighted reduce),
  alpha_src 264:272, alpha_dst 272:280. 768B gather rows (was 1280B f32).
- Replicated dense: every core computes the FULL table with PE matmuls from
  an AllGather'd transposed h (bf16, [272, shard]) -- 2.4x less collective
  wire than gathering the table, and layer 0 needs NO collective at all
  (h0 is host-replicated input).
- Collectives chunked 4x per layer and issued mid-edge-phase so they (and
  the dense of the next layer) overlap with edge-phase gather/DVE work.
- Greedy balanced A/B source split (per-dst in-degree discrepancy
  minimization) cuts ELL padding 1.31x -> 1.22x.
- Edge phase fully bf16: 2x DVE via packed ops; weighted slot-sum via an
  in-place halving tree (all levels 2x mode), final combine in f32.
- Own-tile alpha_dst fetched via two 128-row gathers (A and B tables, one
  is dummy) since SPMD cores can't address "own" rows of a replicated
  table statically.

kernel(**inputs) takes full unsharded inputs, returns full [50000, 8] f32.
"""

import math
import os
import numpy as np
import ml_dtypes

os.environ.setdefault("NEURON_RT_RESET_CORES", "1")

import concourse.bacc as bacc
import concourse.mybir as mybir
import concourse.tile as tile
from concourse.masks import make_identity

bf16 = ml_dtypes.bfloat16
N_CORES = 8
NT = 49
L = 128 * NT
PB = L + 8
NROWS = N_CORES * PB
HALF = 4 * PB
NSLOT = N_CORES * L
HS = NSLOT // 2
NH = 8
HC = 256
CT = 384          # table row elements (bf16) = 768 B
CW = 280          # table cols actually computed/written
F0 = 144          # layer-0 h rows (x 0:128 | qY 128:136 | one 136 | pad)
FH = 272          # layer 1/2 h rows (Hcm 0:256 | qY 256:264 | one 264 | pad)
CMAX = 28
F32 = mybir.dt.float32
BF16 = mybir.dt.bfloat16
I16 = mybir.dt.int16
AF = mybir.ActivationFunctionType
OP = mybir.AluOpType

# c-major permutation: cm col j holds std channel (j%8)*32 + j//8
_J = np.arange(HC)
STDCOL = (_J % 8) * 32 + _J // 8

# chunk boundaries (tiles) for the AllGather/dense pipeline
CHB = [0, 12, 24, 36, NT]
NCH = 4


# ----------------------------------------------------------------------------
# host preprocessing
# ----------------------------------------------------------------------------
def greedy_half(src, dst, n):
    """Assign each node to table half A(False)/B(True), balancing per-dst
    in-degree split, respecting half capacity HS."""
    order = np.argsort(src, kind="stable")
    ds = dst[order]
    counts = np.bincount(src, minlength=n)
    starts = np.r_[0, np.cumsum(counts)]
    imb = np.zeros(n, np.int32)
    half = np.zeros(n, bool)
    capA = capB = 0
    for s in np.argsort(-counts, kind="stable"):
        nb = ds[starts[s]:starts[s + 1]]
        sc = int(imb[nb].sum())
        toB = sc > 0 or (sc == 0 and capB < capA)
        if toB and capB >= HS:
            toB = False
        if (not toB) and capA >= HS:
            toB = True
        if toB:
            imb[nb] -= 1
            capB += 1
            half[s] = True
        else:
            imb[nb] += 1
            capA += 1
    return half


def preprocess(adj, n, cmax=CMAX):
    src = np.concatenate([adj[0], np.arange(n)]).astype(np.int64)
    dst = np.concatenate([adj[1], np.arange(n)]).astype(np.int64)
    half_bit = greedy_half(src, dst, n)

    src_is_b = half_bit[src]
    degA = np.bincount(dst[~src_is_b], minlength=n)
    degB = np.bincount(dst[src_is_b], minlength=n)
    deg = degA + degB

    A_nodes = np.flatnonzero(~half_bit)
    B_nodes = np.flatnonzero(half_bit)
    assert len(A_nodes) <= HS and len(B_nodes) <= HS
    A_sorted = A_nodes[np.lexsort((degA[A_nodes], deg[A_nodes]))]
    B_sorted = B_nodes[np.lexsort((degA[B_nodes], deg[B_nodes]))]
    A_list = np.concatenate([np.full(HS - len(A_sorted), -1, np.int64), A_sorted])
    B_list = np.concatenate([np.full(HS - len(B_sorted), -1, np.int64), B_sorted])

    slots = np.full(NSLOT, -1, dtype=np.int64)
    r = np.arange(NSLOT)
    t = r // 128
    k = t % N_CORES
    i = t // N_CORES
    p = r % 128
    jA = i * 4 + k
    jB = i * 4 + (k - 4)
    selA = k < 4
    slots[selA] = A_list[jA[selA] * 128 + p[selA]]
    slots[~selA] = B_list[jB[~selA] * 128 + p[~selA]]
    physrow = k * PB + i * 128 + p
    node2phys = np.full(n, -1, dtype=np.int64)
    real = slots >= 0
    node2phys[slots[real]] = physrow[real]
    assert (node2phys >= 0).all()
    assert (node2phys[A_nodes] < HALF).all()
    assert (node2phys[B_nodes] >= HALF).all()

    dphys = node2phys[dst]
    dk = dphys // PB
    dloc = dphys % PB
    di = dloc // 128
    dp = dloc % 128
    DA = np.zeros((N_CORES, NT), np.int64)
    DB = np.zeros((N_CORES, NT), np.int64)
    node_k = node2phys // PB
    node_i = (node2phys % PB) // 128
    np.maximum.at(DA, (node_k, node_i), degA)
    np.maximum.at(DB, (node_k, node_i), degB)
    DAi = np.maximum(DA.max(axis=0), 1)
    DBi = np.maximum(DB.max(axis=0), 1)

    es = np.lexsort((src, dst))
    ds_, isb_ = dst[es], src_is_b[es]
    dk_, di_, dp_ = dk[es], di[es], dp[es]
    sphys_ = node2phys[src[es]]
    keys = ds_ * 2 + isb_.astype(np.int64)
    sort2 = np.argsort(keys, kind="stable")
    ks_ = keys[sort2]
    starts = np.r_[0, np.flatnonzero(np.diff(ks_)) + 1]
    cum = np.arange(len(ks_))
    seg_start = np.repeat(cum[starts], np.diff(np.r_[starts, len(ks_)]))
    rank = cum - seg_start
    jcol = np.empty(len(ks_), np.int64)
    jcol[sort2] = rank

    sizesA = 128 * DAi
    sizesB = 128 * DBi
    offA = np.concatenate([[0], np.cumsum(sizesA)[:-1]])
    offB = np.concatenate([[0], np.cumsum(sizesB)[:-1]])
    bigA = [np.full(int(sizesA.sum()), L, np.int32) for _ in range(N_CORES)]
    bigB = [np.full(int(sizesB.sum()), L, np.int32) for _ in range(N_CORES)]
    selB = isb_
    for kk in range(N_CORES):
        mA = (~selB) & (dk_ == kk)
        bigA[kk][offA[di_[mA]] + jcol[mA] * 128 + dp_[mA]] = sphys_[mA]
        mB = selB & (dk_ == kk)
        bigB[kk][offB[di_[mB]] + jcol[mB] * 128 + dp_[mB]] = sphys_[mB] - HALF
    coreA = [[bigA[kk][offA[ii]:offA[ii] + sizesA[ii]] for ii in range(NT)]
             for kk in range(N_CORES)]
    coreB = [[bigB[kk][offB[ii]:offB[ii] + sizesB[ii]] for ii in range(NT)]
             for kk in range(N_CORES)]

    ownA = np.zeros((N_CORES, NT, 128), np.int32)
    ownB = np.zeros((N_CORES, NT, 128), np.int32)
    for kk in range(N_CORES):
        for ii in range(NT):
            rows = (kk % 4) * PB + ii * 128 + np.arange(128)
            if kk < 4:
                ownA[kk, ii] = rows
                ownB[kk, ii] = L
            else:
                ownA[kk, ii] = L
                ownB[kk, ii] = rows

    plan = []
    for ii in range(NT):
        ch = []
        for hf, D in (("A", int(DAi[ii])), ("B", int(DBi[ii]))):
            c0 = 0
            while c0 < D:
                cc = min(cmax, D - c0)
                ch.append((hf, c0, cc))
                c0 += cc
        plan.append(ch)

    return dict(slots=slots, node2phys=node2phys, DAi=DAi, DBi=DBi,
                coreA=coreA, coreB=coreB, ownA=ownA, ownB=ownB, plan=plan,
                half_bit=half_bit, n=n)


def wrap_idx(block_i32):
    num = block_i32.shape[0]
    assert num % 16 == 0
    g = block_i32.reshape(num // 16, 16).T.astype(np.int16)
    return np.tile(g, (8, 1))  # [128, num/16]


def build_weights(inputs):
    W = [np.asarray(inputs[f"W{i}"], np.float32) for i in range(3)]
    att_src = np.asarray(inputs["att_src"], np.float32)
    att_dst = np.asarray(inputs["att_dst"], np.float32)
    bias = np.asarray(inputs["bias"], np.float32)

    Whats = []
    for l in range(3):
        if l == 0:
            Fh, one_row, nrm = F0, 136, 136
            rowmap = np.arange(136)
        else:
            Fh, one_row, nrm = FH, 264, 264
            rowmap = np.concatenate([STDCOL, np.arange(256, 264)])
        As = np.zeros((HC, NH), np.float32)
        Ad = np.zeros((HC, NH), np.float32)
        for hh in range(NH):
            As[hh * 32:(hh + 1) * 32, hh] = att_src[l, hh]
            Ad[hh * 32:(hh + 1) * 32, hh] = att_dst[l, hh]
        WAs = W[l] @ As
        WAd = W[l] @ Ad
        What = np.zeros((Fh, CT), np.float32)
        What[:nrm, 0:HC] = W[l][rowmap][:, STDCOL]
        What[one_row, HC:HC + NH] = 1.0
        What[:nrm, 264:272] = WAs[rowmap]
        What[:nrm, 272:280] = WAd[rowmap]
        Whats.append(np.ascontiguousarray(What.astype(bf16)))

    bias_cm = bias[:, STDCOL]
    fin_w1 = np.asarray(inputs["fin_w1"], np.float32)
    fw1 = np.zeros((FH, 528), np.float32)
    fw1[0:256] = fin_w1[STDCOL]
    fw1[256:264] = fin_w1[256:264]
    tmlp_w2_cm = np.ascontiguousarray(np.asarray(inputs["tmlp_w2"], np.float32)[:, STDCOL])
    tmlp_b2_cm = np.ascontiguousarray(np.asarray(inputs["tmlp_b2"], np.float32)[STDCOL])
    return Whats, bias_cm, np.ascontiguousarray(fw1.astype(bf16)), tmlp_w2_cm, tmlp_b2_cm


def host_inputs(inputs, prep):
    n = prep["n"]
    x = np.asarray(inputs["x"], np.float32)
    qY = np.asarray(inputs["q_Y_sample"], np.float32)
    Whats, bias_cm, fw1, tmlp_w2_cm, tmlp_b2_cm = build_weights(inputs)

    slots = prep["slots"]
    real = slots >= 0
    r = np.arange(NSLOT)
    t = r // 128
    k = t % N_CORES
    i = t // N_CORES
    p = r % 128
    phys = k * PB + i * 128 + p

    # hrepT0 [8*F0, PB] bf16: block k, row f, col = i*128+p (local); empty/dummy cols 0
    kk_ = phys[real] // PB
    loc_ = phys[real] % PB
    nodes = slots[real]
    hT = np.zeros((N_CORES, F0, PB), np.float32)
    xq = np.concatenate([x[nodes], qY[nodes]], axis=1)  # [nreal, 136]
    hT[kk_[:, None], np.arange(136)[None, :], loc_[:, None]] = xq
    hT[kk_, 136, loc_] = 1.0
    hrepT0 = np.ascontiguousarray(hT.reshape(N_CORES * F0, PB).astype(bf16))

    dumrow = np.zeros((8, CT), np.float32)
    dumrow[:, 264:272] = -1e4
    onepad = np.zeros((128, 8), np.float32)
    onepad[:, 0] = 1.0

    half = 64
    freqs4 = np.exp(np.arange(half, dtype=np.float32)
                    * (-math.log(10000.0) / (half - 1))).astype(np.float32)
    b_repcm = np.stack([np.tile(bias_cm[l][None, :], (128, 1)) for l in range(3)])
    fin_b1 = np.asarray(inputs["fin_b1"], np.float32)
    fin_b2 = np.asarray(inputs["fin_b2"], np.float32)
    b2c = tmlp_b2_cm.reshape(256, 1)

    common = {
        "What0": Whats[0], "What1": Whats[1], "What2": Whats[2],
        "b_repcm": b_repcm.astype(np.float32),
        "fin_w1b": fw1,
        "fin_w2b": np.asarray(inputs["fin_w2"], np.float32).astype(bf16),
        "fin_b1rep": np.tile(fin_b1[None, :], (128, 1)).astype(np.float32),
        "fin_b2rep": np.tile(fin_b2[None, :], (128, 1)).astype(np.float32),
        "tmlp_w1": np.asarray(inputs["tmlp_w1"], np.float32),
        "tmlp_b1col": np.asarray(inputs["tmlp_b1"], np.float32).reshape(128, 1),
        "tmlp_w2cm": tmlp_w2_cm,
        "tmlp_b2cols": np.concatenate([b2c[:128], b2c[128:]], axis=1).astype(np.float32),
        "freqs4": freqs4.reshape(half, 1),
        "t_in": np.asarray(inputs["t"], np.float32).reshape(1, 1),
        "dumrow": dumrow.astype(bf16),
        "onepad": onepad,
        "hrepT0": hrepT0,
    }

    in_maps = []
    for kk in range(N_CORES):
        cols = []
        for ii in range(NT):
            cols.append(wrap_idx(prep["ownA"][kk, ii]))
            cols.append(wrap_idx(prep["ownB"][kk, ii]))
            for (hf, c0, cc) in prep["plan"][ii]:
                blk = prep["coreA"][kk][ii] if hf == "A" else prep["coreB"][kk][ii]
                cols.append(wrap_idx(blk[c0 * 128:(c0 + cc) * 128]))
        idx_all = np.ascontiguousarray(np.concatenate(cols, axis=1))
        # qY in slot order for this core
        qYs = np.zeros((L, NH), np.float32)
        sel = (k == kk) & real
        qYs[i[sel] * 128 + p[sel]] = qY[slots[sel]]
        m = dict(common)
        m["qY_shard"] = qYs
        m["idx_all"] = idx_all
        in_maps.append(m)
    return in_maps


# ----------------------------------------------------------------------------
# bass program
# ----------------------------------------------------------------------------
def build_program(prep):
    plan = prep["plan"]
    IDXC = sum(16 + sum(cc * 8 for (_, _, cc) in plan[ii]) for ii in range(NT))
    max_tile_idxc = max(16 + sum(cc * 8 for (_, _, cc) in plan[ii]) for ii in range(NT))

    nc = bacc.Bacc("TRN2", target_bir_lowering=False, debug=False,
                   enable_asserts=False, num_devices=N_CORES)

    # ---- inputs
    hrepT0 = nc.dram_tensor("hrepT0", [N_CORES * F0, PB], BF16, kind="ExternalInput")
    qYs_d = nc.dram_tensor("qY_shard", [L, NH], F32, kind="ExternalInput")
    idx_all = nc.dram_tensor("idx_all", [128, IDXC], I16, kind="ExternalInput")
    What = [nc.dram_tensor(f"What{l}", [F0 if l == 0 else FH, CT], BF16,
                           kind="ExternalInput") for l in range(3)]
    b_repcm = nc.dram_tensor("b_repcm", [3, 128, HC], F32, kind="ExternalInput")
    fin_w1b = nc.dram_tensor("fin_w1b", [FH, 528], BF16, kind="ExternalInput")
    fin_w2b = nc.dram_tensor("fin_w2b", [528, NH], BF16, kind="ExternalInput")
    fin_b1rep = nc.dram_tensor("fin_b1rep", [128, 528], F32, kind="ExternalInput")
    fin_b2rep = nc.dram_tensor("fin_b2rep", [128, NH], F32, kind="ExternalInput")
    tw1 = nc.dram_tensor("tmlp_w1", [128, 128], F32, kind="ExternalInput")
    tb1c = nc.dram_tensor("tmlp_b1col", [128, 1], F32, kind="ExternalInput")
    tw2 = nc.dram_tensor("tmlp_w2cm", [128, HC], F32, kind="ExternalInput")
    tb2c = nc.dram_tensor("tmlp_b2cols", [128, 2], F32, kind="ExternalInput")
    freqs4 = nc.dram_tensor("freqs4", [64, 1], F32, kind="ExternalInput")
    t_in = nc.dram_tensor("t_in", [1, 1], F32, kind="ExternalInput")
    dumrow = nc.dram_tensor("dumrow", [8, CT], BF16, kind="ExternalInput")
    onepad = nc.dram_tensor("onepad", [128, 8], F32, kind="ExternalInput")

    out = nc.dram_tensor("out", [L, NH], F32, kind="ExternalOutput")

    # ---- internals
    T = [nc.dram_tensor(f"table{l}", [NROWS, CT], BF16, kind="Internal")
         for l in range(3)]
    chcols = [(CHB[c + 1] - CHB[c]) * 128 for c in range(NCH)]
    hTc = {}
    hrepTc = {}
    for l in (1, 2):
        hTc[l] = [nc.dram_tensor(f"hT{l}_{c}", [FH, chcols[c]], BF16, kind="Internal")
                  for c in range(NCH)]
        hrepTc[l] = [nc.dram_tensor(f"hrepT{l}_{c}", [N_CORES * FH, chcols[c]], BF16,
                                    kind="Internal", addr_space="Shared")
                     for c in range(NCH)]

    with tile.TileContext(nc) as tc:
        import contextlib
        with contextlib.ExitStack() as ctx:
            consts = ctx.enter_context(tc.tile_pool(name="consts", bufs=1))
            sb = ctx.enter_context(tc.tile_pool(name="sb", bufs=3))
            sb2 = ctx.enter_context(tc.tile_pool(name="sb2", bufs=2))
            isb = ctx.enter_context(tc.tile_pool(name="isb", bufs=2))
            gp = ctx.enter_context(tc.tile_pool(name="gp", bufs=2))
            dsb = ctx.enter_context(tc.tile_pool(name="dsb", bufs=2))
            psd = ctx.enter_context(tc.tile_pool(name="psd", bufs=2, space="PSUM"))
            pst = ctx.enter_context(tc.tile_pool(name="pst", bufs=2, space="PSUM"))
            ps1 = ctx.enter_context(tc.tile_pool(name="ps1", bufs=1, space="PSUM"))

            ident = consts.tile([128, 128], F32)
            make_identity(nc, ident[:])

            # ---- temb -> tb[l] tiles [128, 256] f32 (c-major via permuted w2)
            tcol = consts.tile([64, 1], F32, tag="tcol")
            nc.sync.dma_start(out=tcol[0:1, :], in_=t_in[:])
            nc.gpsimd.partition_broadcast(out_ap=tcol[:], in_ap=tcol[0:1, :])
            fq = consts.tile([64, 1], F32, tag="fq")
            nc.sync.dma_start(out=fq[:], in_=freqs4[:])
            xs = consts.tile([64, 1], F32, tag="xs")
            nc.vector.tensor_scalar_mul(xs[:], tcol[:], 4.0)
            ang = consts.tile([64, 1], F32, tag="ang")
            nc.vector.tensor_tensor(out=ang[:], in0=xs[:], in1=fq[:], op=OP.mult)
            TWO_PI = 2 * math.pi
            c1 = float(np.float32(TWO_PI))
            c2 = float(np.float32(TWO_PI - c1))
            c3 = float(TWO_PI - c1 - float(np.float32(TWO_PI - c1)))
            yk = consts.tile([64, 1], F32, tag="yk")
            nc.vector.tensor_scalar_mul(yk[:], ang[:], 1.0 / TWO_PI)
            ki = consts.tile([64, 1], mybir.dt.int32, tag="ki")
            nc.vector.tensor_copy(out=ki[:], in_=yk[:])
            kk_t = consts.tile([64, 1], F32, tag="kk_t")
            nc.vector.tensor_copy(out=kk_t[:], in_=ki[:])
            red = consts.tile([64, 1], F32, tag="red")
            nc.vector.cody_waite_cascade(out=red[:], x=ang[:], k=kk_t[:],
                                         c1=c1, c2=c2, c3=c3)
            rs = consts.tile([64, 1], F32, tag="rs")
            rc = consts.tile([64, 1], F32, tag="rc")
            nc.vector.add_range_wrap(out=rs[:], in_=red[:], shift=0.0,
                                     bound=math.pi, period=TWO_PI)
            nc.vector.add_range_wrap(out=rc[:], in_=red[:], shift=math.pi / 2,
                                     bound=math.pi, period=TWO_PI)
            sc = consts.tile([128, 1], F32, tag="sc")
            sc2 = consts.tile([64, 1], F32, tag="sc2")
            nc.scalar.activation(sc[0:64, :], rs[:], AF.Sin)
            nc.scalar.activation(sc2[:], rc[:], AF.Sin)
            nc.sync.dma_start(out=sc[64:128, :], in_=sc2[:])

            def elu_(xap, tmp_pool, shape, tag):
                e = tmp_pool.tile(shape, F32, tag=tag + "_e")
                rr = tmp_pool.tile(shape, F32, tag=tag + "_r")
                nc.scalar.activation(e[:], xap, AF.Exp)
                nc.vector.tensor_scalar(out=e[:], in0=e[:], scalar1=-1.0,
                                        scalar2=0.0, op0=OP.add, op1=OP.min)
                nc.scalar.activation(rr[:], xap, AF.Relu)
                nc.vector.tensor_tensor(out=xap, in0=e[:], in1=rr[:], op=OP.add)

            tw1_s = consts.tile([128, 128], F32, tag="tw1")
            nc.sync.dma_start(out=tw1_s[:], in_=tw1[:])
            tw2_s = consts.tile([128, HC], F32, tag="tw2")
            nc.sync.dma_start(out=tw2_s[:], in_=tw2[:])
            e1p = ps1.tile([128, 1], F32, tag="tembp")
            nc.tensor.matmul(out=e1p[:], lhsT=tw1_s[:], rhs=sc[:], start=True, stop=True)
            b1c = consts.tile([128, 1], F32, tag="tb1c")
            nc.sync.dma_start(out=b1c[:], in_=tb1c[:])
            e1 = consts.tile([128, 1], F32, tag="e1")
            nc.vector.tensor_tensor(out=e1[:], in0=e1p[:], in1=b1c[:], op=OP.add)
            elu_(e1[:], consts, [128, 1], "elu_temb")
            tcols_p = ps1.tile([128, 2], F32, tag="tembp")
            nc.tensor.matmul(out=tcols_p[:, 0:1], lhsT=tw2_s[:, 0:128], rhs=e1[:],
                             start=True, stop=True)
            nc.tensor.matmul(out=tcols_p[:, 1:2], lhsT=tw2_s[:, 128:256], rhs=e1[:],
                             start=True, stop=True)
            b2c = consts.tile([128, 2], F32, tag="tb2c")
            nc.sync.dma_start(out=b2c[:], in_=tb2c[:])
            tcols = consts.tile([128, 2], F32, tag="tcols")
            nc.vector.tensor_tensor(out=tcols[:], in0=tcols_p[:], in1=b2c[:], op=OP.add)
            trow_p = ps1.tile([2, 128], F32, tag="tembp")
            nc.tensor.transpose(out=trow_p[:], in_=tcols[:], identity=ident[:])
            trow2 = consts.tile([2, 128], F32, tag="trow2")
            nc.scalar.copy(out=trow2[:], in_=trow_p[:])
            trow = consts.tile([1, HC], F32, tag="trow")
            nc.sync.dma_start(out=trow[0:1, 0:128], in_=trow2[0:1, :])
            nc.sync.dma_start(out=trow[0:1, 128:256], in_=trow2[1:2, :])
            temb_rep = consts.tile([128, HC], F32, tag="temb_rep")
            nc.gpsimd.partition_broadcast(out_ap=temb_rep[:], in_ap=trow[:])
            tb = []
            for l in range(3):
                bl = consts.tile([128, HC], F32, tag=f"b_rep{l}")
                nc.sync.dma_start(out=bl[:], in_=b_repcm[l])
                tbl = consts.tile([128, HC], F32, tag=f"tb{l}")
                nc.vector.tensor_tensor(out=tbl[:], in0=temb_rep[:], in1=bl[:], op=OP.add)
                tb.append(tbl)

            # ---- weight tiles
            Wch = []
            for l in range(3):
                F = F0 if l == 0 else FH
                cks = []
                off = 0
                while off < F:
                    kk = min(128, F - off)
                    wt = consts.tile([128, CT], BF16, tag=f"W{l}_{off}")
                    nc.sync.dma_start(out=wt[:kk, :], in_=What[l][off:off + kk, :])
                    cks.append((wt, kk))
                    off += kk
                Wch.append(cks)
            fw1t = []
            off = 0
            while off < FH:
                kk = min(128, FH - off)
                wt = consts.tile([128, 528], BF16, tag=f"fw1_{off}")
                nc.sync.dma_start(out=wt[:kk, :], in_=fin_w1b[off:off + kk, :])
                fw1t.append((wt, kk))
                off += kk
            fw2t = []
            off = 0
            while off < 528:
                kk = min(128, 528 - off)
                wt = consts.tile([128, NH], BF16, tag=f"fw2_{off}")
                nc.sync.dma_start(out=wt[:kk, :], in_=fin_w2b[off:off + kk, :])
                fw2t.append((wt, kk))
                off += kk
            fb1 = consts.tile([128, 528], F32, tag="fb1")
            nc.sync.dma_start(out=fb1[:], in_=fin_b1rep[:])
            fb2 = consts.tile([128, NH], F32, tag="fb2")
            nc.sync.dma_start(out=fb2[:], in_=fin_b2rep[:])
            onep = consts.tile([128, 8], F32, tag="onep")
            nc.sync.dma_start(out=onep[:], in_=onepad[:])
            dum_t = consts.tile([8, CT], BF16, tag="dum")
            nc.sync.dma_start(out=dum_t[:], in_=dumrow[:])

            # ---- dense helpers
            def dense_tiles(l, kk_blk, lhs_strips, tile_idx, col_of_tile):
                """Emit matmuls for given tiles of block kk_blk of layer l.
                lhs_strips: list of (tile, rows) for the K chunks.
                tile_idx: iterable of global tile indices i.
                col_of_tile: fn i -> column offset in the strips."""
                for i in tile_idx:
                    c0 = col_of_tile(i)
                    pT = psd.tile([128, CW], F32, tag="pT")
                    ncks = len(lhs_strips)
                    for ci, (st, kk) in enumerate(lhs_strips):
                        nc.tensor.matmul(out=pT[:],
                                         lhsT=st[:kk, c0:c0 + 128],
                                         rhs=Wch[l][ci][0][:kk, 0:CW],
                                         start=(ci == 0), stop=(ci == ncks - 1))
                    Ts = dsb.tile([128, CW], BF16, tag="Ts")
                    nc.scalar.copy(out=Ts[:], in_=pT[:])
                    nc.sync.dma_start(
                        out=T[l][kk_blk * PB + i * 128:kk_blk * PB + (i + 1) * 128, 0:CW],
                        in_=Ts[:])

            def dense_layer0():
                for kk_blk in range(N_CORES):
                    s0 = dsb.tile([128, L], BF16, tag="ds0a")
                    nc.sync.dma_start(out=s0[:], in_=hrepT0[kk_blk * F0:kk_blk * F0 + 128, 0:L])
                    s1 = dsb.tile([16, L], BF16, tag="ds0b")
                    nc.sync.dma_start(out=s1[:], in_=hrepT0[kk_blk * F0 + 128:kk_blk * F0 + 144, 0:L])
                    dense_tiles(0, kk_blk, [(s0, 128), (s1, 16)],
                                range(NT), lambda i: i * 128)
                for kk_blk in range(N_CORES):
                    nc.sync.dma_start(out=T[0][kk_blk * PB + L:kk_blk * PB + PB, :],
                                      in_=dum_t[:])

            def dense_chunk(l, ch):
                """Dense for layer l (1 or 2) over tile chunk ch."""
                t0, t1 = CHB[ch], CHB[ch + 1]
                for kk_blk in range(N_CORES):
                    strips = []
                    for (r0, kk) in ((0, 128), (128, 128), (256, 16)):
                        st = dsb.tile([128, chcols[NCH - 1]], BF16, tag=f"ds{r0}")
                        nc.sync.dma_start(
                            out=st[:kk, 0:chcols[ch]],
                            in_=hrepTc[l][ch][kk_blk * FH + r0:kk_blk * FH + r0 + kk, :])
                        strips.append((st, kk))
                    dense_tiles(l, kk_blk, strips, range(t0, t1),
                                lambda i: (i - t0) * 128)

            def dumfix(l):
                for kk_blk in range(N_CORES):
                    nc.sync.dma_start(out=T[l][kk_blk * PB + L:kk_blk * PB + PB, :],
                                      in_=dum_t[:])

            # ---- edge phase
            idx_off_by_tile = []
            off = 0
            for ii in range(NT):
                idx_off_by_tile.append(off)
                off += 16 + sum(cc * 8 for (_, _, cc) in plan[ii])

            def edge_tile(l, i):
                TA = T[l][0:HALF, :]
                TB = T[l][HALF:NROWS, :]
                ioff = idx_off_by_tile[i]
                icols = 16 + sum(cc * 8 for (_, _, cc) in plan[i])
                idx_t = isb.tile([128, max_tile_idxc], I16, tag="idx")
                nc.sync.dma_start(out=idx_t[:, 0:icols],
                                  in_=idx_all[:, ioff:ioff + icols])
                gA = gp.tile([128, 1, CT], BF16, tag="gownA")
                nc.gpsimd.dma_gather(out_ap=gA[:, 0:1, :], in_ap=TA,
                                     idxs_ap=idx_t[:, 0:8],
                                     num_idxs=128, num_idxs_reg=128,
                                     elem_size=CT, single_packet=False)
                gB = gp.tile([128, 1, CT], BF16, tag="gownB")
                nc.gpsimd.dma_gather(out_ap=gB[:, 0:1, :], in_ap=TB,
                                     idxs_ap=idx_t[:, 8:16],
                                     num_idxs=128, num_idxs_reg=128,
                                     elem_size=CT, single_packet=False)
                ad_t = sb.tile([128, NH], BF16, tag="ad")
                nc.vector.tensor_tensor(out=ad_t[:], in0=gA[:, 0, 272:280],
                                        in1=gB[:, 0, 272:280], op=OP.add)
                acc = sb.tile([128, 264], F32, tag="acc")
                first = True
                co = 16
                for (hf, c0, cc) in plan[i]:
                    tbl = TA if hf == "A" else TB
                    g_t = gp.tile([128, CMAX, CT], BF16, tag="g")
                    nc.gpsimd.dma_gather(out_ap=g_t[:, 0:cc, :], in_ap=tbl,
                                         idxs_ap=idx_t[:, co:co + cc * 8],
                                         num_idxs=128 * cc, num_idxs_reg=128 * cc,
                                         elem_size=CT, single_packet=False)
                    co += cc * 8
                    lg = sb.tile([128, CMAX, NH], BF16, tag="lg")
                    nc.vector.tensor_tensor(
                        out=lg[:, 0:cc, :], in0=g_t[:, 0:cc, 264:272],
                        in1=ad_t[:].unsqueeze(1).broadcast_to([128, cc, NH]),
                        op=OP.add)
                    nc.vector.scalar_tensor_tensor(
                        out=lg[:, 0:cc, :], in0=lg[:, 0:cc, :], scalar=0.2,
                        in1=lg[:, 0:cc, :], op0=OP.mult, op1=OP.max)
                    w_t = sb.tile([128, CMAX, NH], BF16, tag="w")
                    nc.scalar.activation(w_t[:, 0:cc, :], lg[:, 0:cc, :], AF.Exp)
                    tmp = sb2.tile([128, CMAX, 264], BF16, tag="tmp")
                    gv = g_t[:, 0:cc, 0:264].rearrange("p j (c h) -> p j c h", c=33)
                    wv = w_t[:, 0:cc, :].unsqueeze(2).broadcast_to([128, cc, 33, NH])
                    tv = tmp[:, 0:cc, :].rearrange("p j (c h) -> p j c h", c=33)
                    nc.vector.tensor_tensor(out=tv, in0=gv, in1=wv, op=OP.mult)
                    # in-place halving tree (bf16), final combine f32
                    nn = cc
                    while nn > 2:
                        a = nn // 2
                        nc.vector.tensor_tensor(out=tmp[:, 0:a, :],
                                                in0=tmp[:, 0:a, :],
                                                in1=tmp[:, a:2 * a, :], op=OP.add)
                        if nn % 2:
                            nc.vector.tensor_tensor(out=tmp[:, 0:1, :],
                                                    in0=tmp[:, 0:1, :],
                                                    in1=tmp[:, 2 * a:2 * a + 1, :],
                                                    op=OP.add)
                        nn = a
                    if first:
                        if nn == 2:
                            nc.vector.tensor_tensor(out=acc[:], in0=tmp[:, 0, :],
                                                    in1=tmp[:, 1, :], op=OP.add)
                        else:
                            nc.vector.tensor_copy(out=acc[:], in_=tmp[:, 0, :])
                        first = False
                    else:
                        part = sb.tile([128, 264], F32, tag="part")
                        if nn == 2:
                            nc.vector.tensor_tensor(out=part[:], in0=tmp[:, 0, :],
                                                    in1=tmp[:, 1, :], op=OP.add)
                        else:
                            nc.vector.tensor_copy(out=part[:], in_=tmp[:, 0, :])
                        nc.vector.tensor_tensor(out=acc[:], in0=acc[:], in1=part[:],
                                                op=OP.add)

                # epilogue
                rcp = sb.tile([128, NH], F32, tag="rcp")
                nc.vector.reciprocal(rcp[:], acc[:, 256:264])
                hfull = sb.tile([128, FH], F32, tag="hfull")
                nc.vector.tensor_tensor(
                    out=hfull[:, 0:256].rearrange("p (c h) -> p c h", c=32),
                    in0=acc[:, 0:256].rearrange("p (c h) -> p c h", c=32),
                    in1=rcp[:].unsqueeze(1).broadcast_to([128, 32, NH]),
                    op=OP.mult)
                nc.vector.tensor_tensor(out=hfull[:, 0:256], in0=hfull[:, 0:256],
                                        in1=tb[l][:], op=OP.add)
                elu_(hfull[:, 0:256], sb, [128, 256], "eluh")
                nc.sync.dma_start(out=hfull[:, 256:264],
                                  in_=qYs_d[i * 128:(i + 1) * 128, :])
                nc.scalar.copy(out=hfull[:, 264:272], in_=onep[:])
                if l < 2:
                    ch = min(i // 12, NCH - 1)
                    col0 = (i - CHB[ch]) * 128
                    for (offr, kk) in ((0, 128), (128, 128), (256, 16)):
                        pt = pst.tile([128, 128], F32, tag="pt")
                        nc.tensor.transpose(out=pt[:kk, :],
                                            in_=hfull[:, offr:offr + kk],
                                            identity=ident[:])
                        st = sb.tile([128, 128], BF16, tag=f"st{offr}")
                        nc.scalar.copy(out=st[:kk, :], in_=pt[:kk, :])
                        nc.sync.dma_start(
                            out=hTc[l + 1][ch][offr:offr + kk, col0:col0 + 128],
                            in_=st[:kk, :])
                else:
                    # final MLP
                    hts = []
                    for (offr, kk) in ((0, 128), (128, 128), (256, 16)):
                        pt = pst.tile([128, 128], F32, tag="pt")
                        nc.tensor.transpose(out=pt[:kk, :],
                                            in_=hfull[:, offr:offr + kk],
                                            identity=ident[:])
                        st = sb.tile([128, 128], BF16, tag=f"st{offr}")
                        nc.scalar.copy(out=st[:kk, :], in_=pt[:kk, :])
                        hts.append((st, kk))
                    u = sb.tile([128, 528], F32, tag="u")
                    for half_i in range(2):
                        pm = ps1.tile([128, 264], F32, tag="pmlp")
                        for ci, (st, kk) in enumerate(hts):
                            nc.tensor.matmul(
                                out=pm[:], lhsT=st[:kk, :],
                                rhs=fw1t[ci][0][:kk, half_i * 264:(half_i + 1) * 264],
                                start=(ci == 0), stop=(ci == 2))
                        nc.vector.tensor_tensor(
                            out=u[:, half_i * 264:(half_i + 1) * 264],
                            in0=pm[:], in1=fb1[:, half_i * 264:(half_i + 1) * 264],
                            op=OP.add)
                    elu_(u[:], sb, [128, 528], "elu_u")
                    po = ps1.tile([128, NH], F32, tag="po")
                    for ci in range(5):
                        offc = ci * 128
                        kk = min(128, 528 - offc)
                        pt = pst.tile([128, 128], F32, tag="pt")
                        nc.tensor.transpose(out=pt[:kk, :],
                                            in_=u[:, offc:offc + kk],
                                            identity=ident[:])
                        st = sb.tile([128, 128], BF16, tag="uT")
                        nc.scalar.copy(out=st[:kk, :], in_=pt[:kk, :])
                        nc.tensor.matmul(out=po[:], lhsT=st[:kk, :],
                                         rhs=fw2t[ci][0][:kk, :],
                                         start=(ci == 0), stop=(ci == 4))
                    o_t = sb.tile([128, NH], F32, tag="o_t")
                    nc.vector.tensor_tensor(out=o_t[:], in0=po[:], in1=fb2[:],
                                            op=OP.add)
                    nc.sync.dma_start(out=out[i * 128:(i + 1) * 128, :], in_=o_t[:])

            RG = [list(range(N_CORES))]

            # ---- layer 0
            dense_layer0()
            for i in range(NT):
                edge_tile(0, i)
                if i + 1 in CHB[1:]:
                    ch = CHB[1:].index(i + 1)
                    nc.gpsimd.collective_compute(
                        "AllGather", OP.bypass, replica_groups=RG,
                        ins=[hTc[1][ch][:]], outs=[hrepTc[1][ch][:]])
                    dense_chunk(1, ch)
            dumfix(1)
            # ---- layer 1
            for i in range(NT):
                edge_tile(1, i)
                if i + 1 in CHB[1:]:
                    ch = CHB[1:].index(i + 1)
                    nc.gpsimd.collective_compute(
                        "AllGather", OP.bypass, replica_groups=RG,
                        ins=[hTc[2][ch][:]], outs=[hrepTc[2][ch][:]])
                    dense_chunk(2, ch)
            dumfix(2)
            # ---- layer 2 + MLP
            for i in range(NT):
                edge_tile(2, i)

    nc.compile()
    return nc


def run(inputs, trace=False):
    from concourse.bass_utils import run_bass_kernel_spmd
    from concourse.bass_interp import get_hw_module
    adj = np.asarray(inputs["adj"])
    n = int(np.asarray(inputs["x"]).shape[0])
    prep = preprocess(adj, n)
    in_maps = host_inputs(inputs, prep)
    nc = build_program(prep)
    nc.m = get_hw_module(nc.m)
    res = run_bass_kernel_spmd(nc, in_maps, core_ids=list(range(N_CORES)),
                               trace=trace)
    outs = [np.asarray(r["out"]) for r in res.results]
    y_slots = np.zeros((NSLOT, NH), np.float32)
    for k in range(N_CORES):
        for i in range(NT):
            slot_base = (i * N_CORES + k) * 128
            y_slots[slot_base:slot_base + 128] = outs[k][i * 128:(i + 1) * 128]
    slots = prep["slots"]
    r_real = np.flatnonzero(slots >= 0)
    y = np.zeros((n, NH), np.float32)
    y[slots[r_real]] = y_slots[r_real]
    return y, res


def kernel(**inputs) -> np.ndarray:
    y, _ = run(inputs)
    return y


# revision 20
# speedup vs baseline: 2.8662x; 1.7370x over previous
"""Trainium2 Bass kernel for the 3-layer GAT denoising model
(nn_Denoising_Model_24764781429262): N=50000 nodes, E=800000 edges, 8 heads.

Design:
- bf16 table [NROWS, 384]: H in c-major layout (cols 0:256), ones cols
  256:264 (softmax denominator folded into the main weighted reduce),
  alpha_src 264:272, alpha_dst 272:280. 768B gather rows.
- Replicated dense: every core computes the FULL table with PE matmuls from
  an AllGather'd transposed h (bf16) -- layer 0 needs NO collective at all
  (h0 is host-replicated input).
- Collectives tapered-chunked per layer (big chunks early so they hide
  under the edge phase, small final chunk to minimize the exposed tail)
  and issued from the PE queue so they never block the gather-critical
  Pool queue; deep hfull buffering lets DVE run ahead during the block.
- Greedy balanced A/B source split minimizes ELL padding; self-loop edges
  are forced to slot 0 of their half so the own-tile alpha_dst comes from
  the main gathers (mask-select) with no extra gather instructions.
- Edge phase fully bf16 (packed 2x DVE); weighted slot-sum via in-place
  halving tree, final combine f32.

kernel(**inputs) takes full unsharded inputs, returns full [50000, 8] f32.
"""

import math
import os
import numpy as np
import ml_dtypes

os.environ.setdefault("NEURON_RT_RESET_CORES", "1")

import concourse.bacc as bacc
import concourse.bass as cbass
import concourse.mybir as mybir
import concourse.tile as tile
from concourse.masks import make_identity

bf16 = ml_dtypes.bfloat16
N_CORES = 8
NT = 49
L = 128 * NT
PB = L + 8
NROWS = N_CORES * PB
HALF = 4 * PB
NSLOT = N_CORES * L
HS = NSLOT // 2
NH = 8
HC = 256
CT = 384          # table row elements (bf16) = 768 B
CW = 280          # table cols actually computed/written
F0 = 144          # layer-0 h rows (x 0:128 | qY 128:136 | one 136 | pad)
FH = 272          # layer 1/2 h rows (Hcm 0:256 | qY 256:264 | one 264 | pad)
CMAX = 20
F32 = mybir.dt.float32
BF16 = mybir.dt.bfloat16
I16 = mybir.dt.int16
AF = mybir.ActivationFunctionType
OP = mybir.AluOpType

# c-major permutation: cm col j holds std channel (j%8)*32 + j//8
_J = np.arange(HC)
STDCOL = (_J % 8) * 32 + _J // 8

# AllGather/dense pipeline chunk boundaries (tiles); tapered so the final
# exposed chunk is small.
CHB = [0, 16, 32, 44, NT]
AG_ENGINE = os.environ.get("AG_ENGINE", "pool")


def chunk_of_tile(i):
    for c in range(len(CHB) - 1):
        if i < CHB[c + 1]:
            return c
    raise AssertionError


# ----------------------------------------------------------------------------
# host preprocessing
# ----------------------------------------------------------------------------
def greedy_half(src, dst, n):
    order = np.argsort(src, kind="stable")
    ds = dst[order]
    counts = np.bincount(src, minlength=n)
    starts = np.r_[0, np.cumsum(counts)]
    imb = np.zeros(n, np.int32)
    half = np.zeros(n, bool)
    capA = capB = 0
    for s in np.argsort(-counts, kind="stable"):
        nb = ds[starts[s]:starts[s + 1]]
        sc = int(imb[nb].sum())
        toB = sc > 0 or (sc == 0 and capB < capA)
        if toB and capB >= HS:
            toB = False
        if (not toB) and capA >= HS:
            toB = True
        if toB:
            imb[nb] -= 1
            capB += 1
            half[s] = True
        else:
            imb[nb] += 1
            capA += 1
    return half


def preprocess(adj, n, cmax=CMAX):
    src = np.concatenate([adj[0], np.arange(n)]).astype(np.int64)
    dst = np.concatenate([adj[1], np.arange(n)]).astype(np.int64)
    half_bit = greedy_half(src, dst, n)

    src_is_b = half_bit[src]
    degA = np.bincount(dst[~src_is_b], minlength=n)
    degB = np.bincount(dst[src_is_b], minlength=n)
    deg = degA + degB

    A_nodes = np.flatnonzero(~half_bit)
    B_nodes = np.flatnonzero(half_bit)
    assert len(A_nodes) <= HS and len(B_nodes) <= HS
    A_sorted = A_nodes[np.lexsort((degA[A_nodes], deg[A_nodes]))]
    B_sorted = B_nodes[np.lexsort((degA[B_nodes], deg[B_nodes]))]
    A_list = np.concatenate([np.full(HS - len(A_sorted), -1, np.int64), A_sorted])
    B_list = np.concatenate([np.full(HS - len(B_sorted), -1, np.int64), B_sorted])

    slots = np.full(NSLOT, -1, dtype=np.int64)
    r = np.arange(NSLOT)
    t = r // 128
    k = t % N_CORES
    i = t // N_CORES
    p = r % 128
    jA = i * 4 + k
    jB = i * 4 + (k - 4)
    selA = k < 4
    slots[selA] = A_list[jA[selA] * 128 + p[selA]]
    slots[~selA] = B_list[jB[~selA] * 128 + p[~selA]]
    physrow = k * PB + i * 128 + p
    node2phys = np.full(n, -1, dtype=np.int64)
    real = slots >= 0
    node2phys[slots[real]] = physrow[real]
    assert (node2phys >= 0).all()
    assert (node2phys[A_nodes] < HALF).all()
    assert (node2phys[B_nodes] >= HALF).all()

    dphys = node2phys[dst]
    dk = dphys // PB
    dloc = dphys % PB
    di = dloc // 128
    dp = dloc % 128
    DA = np.zeros((N_CORES, NT), np.int64)
    DB = np.zeros((N_CORES, NT), np.int64)
    node_k = node2phys // PB
    node_i = (node2phys % PB) // 128
    np.maximum.at(DA, (node_k, node_i), degA)
    np.maximum.at(DB, (node_k, node_i), degB)
    DAi = np.maximum(DA.max(axis=0), 1)
    DBi = np.maximum(DB.max(axis=0), 1)

    # per-(dst, half) rank, self-loop forced to rank 0 of its half
    not_self = (src != dst).astype(np.int64)
    es = np.lexsort((src, not_self, dst))
    ds_, isb_ = dst[es], src_is_b[es]
    dk_, di_, dp_ = dk[es], di[es], dp[es]
    sphys_ = node2phys[src[es]]
    keys = ds_ * 2 + isb_.astype(np.int64)
    sort2 = np.argsort(keys, kind="stable")
    ks_ = keys[sort2]
    starts = np.r_[0, np.flatnonzero(np.diff(ks_)) + 1]
    cum = np.arange(len(ks_))
    seg_start = np.repeat(cum[starts], np.diff(np.r_[starts, len(ks_)]))
    rank = cum - seg_start
    jcol = np.empty(len(ks_), np.int64)
    jcol[sort2] = rank

    sizesA = 128 * DAi
    sizesB = 128 * DBi
    offA = np.concatenate([[0], np.cumsum(sizesA)[:-1]])
    offB = np.concatenate([[0], np.cumsum(sizesB)[:-1]])
    bigA = [np.full(int(sizesA.sum()), L, np.int32) for _ in range(N_CORES)]
    bigB = [np.full(int(sizesB.sum()), L, np.int32) for _ in range(N_CORES)]
    selB = isb_
    for kk in range(N_CORES):
        mA = (~selB) & (dk_ == kk)
        bigA[kk][offA[di_[mA]] + jcol[mA] * 128 + dp_[mA]] = sphys_[mA]
        mB = selB & (dk_ == kk)
        bigB[kk][offB[di_[mB]] + jcol[mB] * 128 + dp_[mB]] = sphys_[mB] - HALF
    coreA = [[bigA[kk][offA[ii]:offA[ii] + sizesA[ii]] for ii in range(NT)]
             for kk in range(N_CORES)]
    coreB = [[bigB[kk][offB[ii]:offB[ii] + sizesB[ii]] for ii in range(NT)]
             for kk in range(N_CORES)]

    # sanity: self-loop of dst (k,i,p) sits at col 0 of its half-block
    plan = []
    for ii in range(NT):
        ch = []
        for hf, D in (("A", int(DAi[ii])), ("B", int(DBi[ii]))):
            c0 = 0
            while c0 < D:
                cc = min(cmax, D - c0)
                ch.append((hf, c0, cc))
                c0 += cc
        plan.append(ch)

    # per-core dst-half masks [128, NT]
    maskA = np.zeros((N_CORES, 128, NT), np.float32)
    for kk in range(N_CORES):
        if kk < 4:
            maskA[kk, :, :] = 1.0
    # (dst half == core block half: cores 0..3 hold A-half dsts)

    return dict(slots=slots, node2phys=node2phys, DAi=DAi, DBi=DBi,
                coreA=coreA, coreB=coreB, plan=plan, maskA=maskA,
                half_bit=half_bit, n=n)


def wrap_idx(block_i32):
    num = block_i32.shape[0]
    assert num % 16 == 0
    g = block_i32.reshape(num // 16, 16).T.astype(np.int16)
    return np.tile(g, (8, 1))


def build_weights(inputs):
    W = [np.asarray(inputs[f"W{i}"], np.float32) for i in range(3)]
    att_src = np.asarray(inputs["att_src"], np.float32)
    att_dst = np.asarray(inputs["att_dst"], np.float32)
    bias = np.asarray(inputs["bias"], np.float32)

    Whats = []
    for l in range(3):
        if l == 0:
            Fh, one_row, nrm = F0, 136, 136
            rowmap = np.arange(136)
        else:
            Fh, one_row, nrm = FH, 264, 264
            rowmap = np.concatenate([STDCOL, np.arange(256, 264)])
        As = np.zeros((HC, NH), np.float32)
        Ad = np.zeros((HC, NH), np.float32)
        for hh in range(NH):
            As[hh * 32:(hh + 1) * 32, hh] = att_src[l, hh]
            Ad[hh * 32:(hh + 1) * 32, hh] = att_dst[l, hh]
        WAs = W[l] @ As
        WAd = W[l] @ Ad
        What = np.zeros((Fh, CT), np.float32)
        What[:nrm, 0:HC] = W[l][rowmap][:, STDCOL]
        What[one_row, HC:HC + NH] = 1.0
        What[:nrm, 264:272] = WAs[rowmap]
        What[:nrm, 272:280] = WAd[rowmap]
        Whats.append(np.ascontiguousarray(What.astype(bf16)))

    bias_cm = bias[:, STDCOL]
    fin_w1 = np.asarray(inputs["fin_w1"], np.float32)
    fw1 = np.zeros((FH, 528), np.float32)
    fw1[0:256] = fin_w1[STDCOL]
    fw1[256:264] = fin_w1[256:264]
    tmlp_w2_cm = np.ascontiguousarray(np.asarray(inputs["tmlp_w2"], np.float32)[:, STDCOL])
    tmlp_b2_cm = np.ascontiguousarray(np.asarray(inputs["tmlp_b2"], np.float32)[STDCOL])
    return Whats, bias_cm, np.ascontiguousarray(fw1.astype(bf16)), tmlp_w2_cm, tmlp_b2_cm


def host_inputs(inputs, prep):
    n = prep["n"]
    x = np.asarray(inputs["x"], np.float32)
    qY = np.asarray(inputs["q_Y_sample"], np.float32)
    Whats, bias_cm, fw1, tmlp_w2_cm, tmlp_b2_cm = build_weights(inputs)

    slots = prep["slots"]
    real = slots >= 0
    r = np.arange(NSLOT)
    t = r // 128
    k = t % N_CORES
    i = t // N_CORES
    p = r % 128
    phys = k * PB + i * 128 + p

    kk_ = phys[real] // PB
    loc_ = phys[real] % PB
    nodes = slots[real]
    hT = np.zeros((N_CORES, F0, PB), np.float32)
    xq = np.concatenate([x[nodes], qY[nodes]], axis=1)
    hT[kk_[:, None], np.arange(136)[None, :], loc_[:, None]] = xq
    hT[kk_, 136, loc_] = 1.0
    hrepT0 = np.ascontiguousarray(hT.reshape(N_CORES * F0, PB).astype(bf16))

    dumrow = np.zeros((8, CT), np.float32)
    dumrow[:, 264:272] = -1e4
    onepad = np.zeros((128, 8), np.float32)
    onepad[:, 0] = 1.0

    half = 64
    freqs4 = np.exp(np.arange(half, dtype=np.float32)
                    * (-math.log(10000.0) / (half - 1))).astype(np.float32)
    b_repcm = np.stack([np.tile(bias_cm[l][None, :], (128, 1)) for l in range(3)])
    fin_b1 = np.asarray(inputs["fin_b1"], np.float32)
    fin_b2 = np.asarray(inputs["fin_b2"], np.float32)
    b2c = tmlp_b2_cm.reshape(256, 1)

    common = {
        "What0": Whats[0], "What1": Whats[1], "What2": Whats[2],
        "b_repcm": b_repcm.astype(np.float32),
        "fin_w1b": fw1,
        "fin_w2b": np.asarray(inputs["fin_w2"], np.float32).astype(bf16),
        "fin_b1rep": np.tile(fin_b1[None, :], (128, 1)).astype(np.float32),
        "fin_b2rep": np.tile(fin_b2[None, :], (128, 1)).astype(np.float32),
        "tmlp_w1": np.asarray(inputs["tmlp_w1"], np.float32),
        "tmlp_b1col": np.asarray(inputs["tmlp_b1"], np.float32).reshape(128, 1),
        "tmlp_w2cm": tmlp_w2_cm,
        "tmlp_b2cols": np.concatenate([b2c[:128], b2c[128:]], axis=1).astype(np.float32),
        "freqs4": freqs4.reshape(half, 1),
        "t_in": np.asarray(inputs["t"], np.float32).reshape(1, 1),
        "dumrow": dumrow.astype(bf16),
        "onepad": onepad,
        "hrepT0": hrepT0,
    }

    in_maps = []
    for kk in range(N_CORES):
        cols = []
        for ii in range(NT):
            for (hf, c0, cc) in prep["plan"][ii]:
                blk = prep["coreA"][kk][ii] if hf == "A" else prep["coreB"][kk][ii]
                cols.append(wrap_idx(blk[c0 * 128:(c0 + cc) * 128]))
        idx_all = np.ascontiguousarray(np.concatenate(cols, axis=1))
        # qY in [128, NT*8] layout: qYs[p, i*8+h] = qY[slot(kk, i, p), h]
        qYs = np.zeros((128, NT * NH), np.float32)
        sel = (k == kk) & real
        qYs[p[sel][:, None], (i[sel] * NH)[:, None] + np.arange(NH)[None, :]] = qY[slots[sel]]
        mA = np.ascontiguousarray(prep["maskA"][kk].astype(bf16))
        mB = np.ascontiguousarray((1.0 - prep["maskA"][kk]).astype(bf16))
        m = dict(common)
        m["qY_shard"] = qYs
        m["idx_all"] = idx_all
        m["maskA"] = mA
        m["maskB"] = mB
        in_maps.append(m)
    return in_maps


# ----------------------------------------------------------------------------
# bass program
# ----------------------------------------------------------------------------
def build_program(prep, ag_engine=AG_ENGINE, chb=None):
    if chb is None:
        chb = CHB
    nch = len(chb) - 1
    plan = prep["plan"]
    IDXC = sum(sum(cc * 8 for (_, _, cc) in plan[ii]) for ii in range(NT))

    nc = bacc.Bacc("TRN2", target_bir_lowering=False, debug=False,
                   enable_asserts=False, num_devices=N_CORES)

    hrepT0 = nc.dram_tensor("hrepT0", [N_CORES * F0, PB], BF16, kind="ExternalInput")
    qYs_d = nc.dram_tensor("qY_shard", [128, NT * NH], F32, kind="ExternalInput")
    idx_all = nc.dram_tensor("idx_all", [128, IDXC], I16, kind="ExternalInput")
    What = [nc.dram_tensor(f"What{l}", [F0 if l == 0 else FH, CT], BF16,
                           kind="ExternalInput") for l in range(3)]
    b_repcm = nc.dram_tensor("b_repcm", [3, 128, HC], F32, kind="ExternalInput")
    fin_w1b = nc.dram_tensor("fin_w1b", [FH, 528], BF16, kind="ExternalInput")
    fin_w2b = nc.dram_tensor("fin_w2b", [528, NH], BF16, kind="ExternalInput")
    fin_b1rep = nc.dram_tensor("fin_b1rep", [128, 528], F32, kind="ExternalInput")
    fin_b2rep = nc.dram_tensor("fin_b2rep", [128, NH], F32, kind="ExternalInput")
    tw1 = nc.dram_tensor("tmlp_w1", [128, 128], F32, kind="ExternalInput")
    tb1c = nc.dram_tensor("tmlp_b1col", [128, 1], F32, kind="ExternalInput")
    tw2 = nc.dram_tensor("tmlp_w2cm", [128, HC], F32, kind="ExternalInput")
    tb2c = nc.dram_tensor("tmlp_b2cols", [128, 2], F32, kind="ExternalInput")
    freqs4 = nc.dram_tensor("freqs4", [64, 1], F32, kind="ExternalInput")
    t_in = nc.dram_tensor("t_in", [1, 1], F32, kind="ExternalInput")
    dumrow = nc.dram_tensor("dumrow", [8, CT], BF16, kind="ExternalInput")
    onepad = nc.dram_tensor("onepad", [128, 8], F32, kind="ExternalInput")
    maskA_d = nc.dram_tensor("maskA", [128, NT], BF16, kind="ExternalInput")
    maskB_d = nc.dram_tensor("maskB", [128, NT], BF16, kind="ExternalInput")

    out = nc.dram_tensor("out", [L, NH], F32, kind="ExternalOutput")

    T = [nc.dram_tensor(f"table{l}", [NROWS, CT], BF16, kind="Internal")
         for l in range(3)]
    chcols = [(chb[c + 1] - chb[c]) * 128 for c in range(nch)]
    maxcols = max(chcols)
    hTc = {}
    hrepTc = {}
    for l in (1, 2):
        hTc[l] = [nc.dram_tensor(f"hT{l}_{c}", [FH, chcols[c]], BF16, kind="Internal")
                  for c in range(nch)]
        hrepTc[l] = [nc.dram_tensor(f"hrepT{l}_{c}", [N_CORES * FH, chcols[c]], BF16,
                                    kind="Internal", addr_space="Shared")
                     for c in range(nch)]

    def chunk_of(i):
        for c in range(nch):
            if i < chb[c + 1]:
                return c
        raise AssertionError

    with tile.TileContext(nc) as tc:
        import contextlib
        with contextlib.ExitStack() as ctx:
            consts = ctx.enter_context(tc.tile_pool(name="consts", bufs=1))
            sb = ctx.enter_context(tc.tile_pool(name="sb", bufs=3))
            sb2 = ctx.enter_context(tc.tile_pool(name="sb2", bufs=2))
            hfp = ctx.enter_context(tc.tile_pool(name="hfp", bufs=8))
            stp = ctx.enter_context(tc.tile_pool(name="stp", bufs=6))
            gp = ctx.enter_context(tc.tile_pool(name="gp", bufs=4))
            dsb = ctx.enter_context(tc.tile_pool(
                name="dsb", bufs=2 if maxcols <= 2048 else 1))
            psd = ctx.enter_context(tc.tile_pool(name="psd", bufs=2, space="PSUM"))
            pst = ctx.enter_context(tc.tile_pool(name="pst", bufs=2, space="PSUM"))
            ps1 = ctx.enter_context(tc.tile_pool(name="ps1", bufs=1, space="PSUM"))

            ident = consts.tile([128, 128], F32)
            make_identity(nc, ident[:])

            # ---- temb -> tb[l] [128, 256] f32 (c-major via permuted w2)
            tcol = consts.tile([64, 1], F32, tag="tcol")
            nc.sync.dma_start(out=tcol[0:1, :], in_=t_in[:])
            nc.gpsimd.partition_broadcast(out_ap=tcol[:], in_ap=tcol[0:1, :])
            fq = consts.tile([64, 1], F32, tag="fq")
            nc.sync.dma_start(out=fq[:], in_=freqs4[:])
            xs = consts.tile([64, 1], F32, tag="xs")
            nc.vector.tensor_scalar_mul(xs[:], tcol[:], 4.0)
            ang = consts.tile([64, 1], F32, tag="ang")
            nc.vector.tensor_tensor(out=ang[:], in0=xs[:], in1=fq[:], op=OP.mult)
            TWO_PI = 2 * math.pi
            c1 = float(np.float32(TWO_PI))
            c2 = float(np.float32(TWO_PI - c1))
            c3 = float(TWO_PI - c1 - float(np.float32(TWO_PI - c1)))
            yk = consts.tile([64, 1], F32, tag="yk")
            nc.vector.tensor_scalar_mul(yk[:], ang[:], 1.0 / TWO_PI)
            ki = consts.tile([64, 1], mybir.dt.int32, tag="ki")
            nc.vector.tensor_copy(out=ki[:], in_=yk[:])
            kk_t = consts.tile([64, 1], F32, tag="kk_t")
            nc.vector.tensor_copy(out=kk_t[:], in_=ki[:])
            red = consts.tile([64, 1], F32, tag="red")
            nc.vector.cody_waite_cascade(out=red[:], x=ang[:], k=kk_t[:],
                                         c1=c1, c2=c2, c3=c3)
            rs = consts.tile([64, 1], F32, tag="rs")
            rc = consts.tile([64, 1], F32, tag="rc")
            nc.vector.add_range_wrap(out=rs[:], in_=red[:], shift=0.0,
                                     bound=math.pi, period=TWO_PI)
            nc.vector.add_range_wrap(out=rc[:], in_=red[:], shift=math.pi / 2,
                                     bound=math.pi, period=TWO_PI)
            sc = consts.tile([128, 1], F32, tag="sc")
            sc2 = consts.tile([64, 1], F32, tag="sc2")
            nc.scalar.activation(sc[0:64, :], rs[:], AF.Sin)
            nc.scalar.activation(sc2[:], rc[:], AF.Sin)
            nc.sync.dma_start(out=sc[64:128, :], in_=sc2[:])

            def elu_(xap, tmp_pool, shape, tag):
                # elu(x) = min(exp(x) - 1, relu(x))
                e = tmp_pool.tile(shape, F32, tag=tag + "_e")
                rr = tmp_pool.tile(shape, F32, tag=tag + "_r")
                nc.scalar.activation(e[:], xap, AF.Exp)
                nc.scalar.activation(rr[:], xap, AF.Relu)
                nc.vector.scalar_tensor_tensor(out=xap, in0=e[:], scalar=-1.0,
                                               in1=rr[:], op0=OP.add, op1=OP.min)

            tw1_s = consts.tile([128, 128], F32, tag="tw1")
            nc.sync.dma_start(out=tw1_s[:], in_=tw1[:])
            tw2_s = consts.tile([128, HC], F32, tag="tw2")
            nc.sync.dma_start(out=tw2_s[:], in_=tw2[:])
            e1p = ps1.tile([128, 1], F32, tag="tembp")
            nc.tensor.matmul(out=e1p[:], lhsT=tw1_s[:], rhs=sc[:], start=True, stop=True)
            b1c = consts.tile([128, 1], F32, tag="tb1c")
            nc.sync.dma_start(out=b1c[:], in_=tb1c[:])
            e1 = consts.tile([128, 1], F32, tag="e1")
            nc.vector.tensor_tensor(out=e1[:], in0=e1p[:], in1=b1c[:], op=OP.add)
            elu_(e1[:], consts, [128, 1], "elu_temb")
            tcols_p = ps1.tile([128, 2], F32, tag="tembp")
            nc.tensor.matmul(out=tcols_p[:, 0:1], lhsT=tw2_s[:, 0:128], rhs=e1[:],
                             start=True, stop=True)
            nc.tensor.matmul(out=tcols_p[:, 1:2], lhsT=tw2_s[:, 128:256], rhs=e1[:],
                             start=True, stop=True)
            b2c = consts.tile([128, 2], F32, tag="tb2c")
            nc.sync.dma_start(out=b2c[:], in_=tb2c[:])
            tcols = consts.tile([128, 2], F32, tag="tcols")
            nc.vector.tensor_tensor(out=tcols[:], in0=tcols_p[:], in1=b2c[:], op=OP.add)
            trow_p = ps1.tile([2, 128], F32, tag="tembp")
            nc.tensor.transpose(out=trow_p[:], in_=tcols[:], identity=ident[:])
            trow2 = consts.tile([2, 128], F32, tag="trow2")
            nc.scalar.copy(out=trow2[:], in_=trow_p[:])
            trow = consts.tile([1, HC], F32, tag="trow")
            nc.sync.dma_start(out=trow[0:1, 0:128], in_=trow2[0:1, :])
            nc.sync.dma_start(out=trow[0:1, 128:256], in_=trow2[1:2, :])
            temb_rep = consts.tile([128, HC], F32, tag="temb_rep")
            nc.gpsimd.partition_broadcast(out_ap=temb_rep[:], in_ap=trow[:])
            tb = []
            for l in range(3):
                bl = consts.tile([128, HC], F32, tag=f"b_rep{l}")
                nc.sync.dma_start(out=bl[:], in_=b_repcm[l])
                tbl = consts.tile([128, HC], F32, tag=f"tb{l}")
                nc.vector.tensor_tensor(out=tbl[:], in0=temb_rep[:], in1=bl[:], op=OP.add)
                tb.append(tbl)

            # ---- weights
            Wch = []
            for l in range(3):
                F = F0 if l == 0 else FH
                cks = []
                off = 0
                while off < F:
                    kk = min(128, F - off)
                    wt = consts.tile([128, CT], BF16, tag=f"W{l}_{off}")
                    nc.sync.dma_start(out=wt[:kk, :], in_=What[l][off:off + kk, :])
                    cks.append((wt, kk))
                    off += kk
                Wch.append(cks)
            fw1t = []
            off = 0
            while off < FH:
                kk = min(128, FH - off)
                wt = consts.tile([128, 528], BF16, tag=f"fw1_{off}")
                nc.sync.dma_start(out=wt[:kk, :], in_=fin_w1b[off:off + kk, :])
                fw1t.append((wt, kk))
                off += kk
            fw2t = []
            off = 0
            while off < 528:
                kk = min(128, 528 - off)
                wt = consts.tile([128, NH], BF16, tag=f"fw2_{off}")
                nc.sync.dma_start(out=wt[:kk, :], in_=fin_w2b[off:off + kk, :])
                fw2t.append((wt, kk))
                off += kk
            fb1 = consts.tile([128, 528], F32, tag="fb1")
            nc.sync.dma_start(out=fb1[:], in_=fin_b1rep[:])
            fb2 = consts.tile([128, NH], F32, tag="fb2")
            nc.sync.dma_start(out=fb2[:], in_=fin_b2rep[:])
            onep = consts.tile([128, 8], F32, tag="onep")
            nc.sync.dma_start(out=onep[:], in_=onepad[:])
            dum_t = consts.tile([8, CT], BF16, tag="dum")
            nc.sync.dma_start(out=dum_t[:], in_=dumrow[:])
            mA_s = consts.tile([128, NT], BF16, tag="mA")
            nc.sync.dma_start(out=mA_s[:], in_=maskA_d[:])
            mB_s = consts.tile([128, NT], BF16, tag="mB")
            nc.sync.dma_start(out=mB_s[:], in_=maskB_d[:])
            qYs_s = consts.tile([128, NT * NH], F32, tag="qYs")
            nc.sync.dma_start(out=qYs_s[:], in_=qYs_d[:])
            idx_s = consts.tile([128, IDXC], I16, tag="idx_s")
            nc.sync.dma_start(out=idx_s[:], in_=idx_all[:])

            # ---- dense helpers (batched table writes, 4 tiles per DMA)
            def dense_range(l, kk_blk, lhs_strips, tiles, col_of_tile):
                group = []

                def flush():
                    if not group:
                        return
                    g = len(group)
                    i0 = group[0][1]
                    Ts4 = dsb.tile([128, 2, CW], BF16, tag="Ts4")
                    for (j, (pT, _)) in enumerate(group):
                        nc.scalar.copy(out=Ts4[:, j, :], in_=pT[:])
                    dst = T[l][kk_blk * PB + i0 * 128:
                               kk_blk * PB + (i0 + g) * 128, 0:CW]
                    nc.sync.dma_start(
                        out=dst.rearrange("(t p) c -> p t c", t=g),
                        in_=Ts4[:, 0:g, :])
                    group.clear()

                for i in tiles:
                    c0 = col_of_tile(i)
                    pT = psd.tile([128, CW], F32, tag="pT")
                    ncks = len(lhs_strips)
                    for ci, (st, kk) in enumerate(lhs_strips):
                        nc.tensor.matmul(out=pT[:],
                                         lhsT=st[:kk, c0:c0 + 128],
                                         rhs=Wch[l][ci][0][:kk, 0:CW],
                                         start=(ci == 0), stop=(ci == ncks - 1))
                    group.append((pT, i))
                    if len(group) == 2:
                        flush()
                flush()

            def dense_chunk(l, ch):
                t0, t1 = chb[ch], chb[ch + 1]
                for kk_blk in range(N_CORES):
                    strips = []
                    if l == 0:
                        srcrows = [(0, 128), (128, 16)]
                        base = kk_blk * F0
                        srct = hrepT0
                        colrange = (t0 * 128, t1 * 128)
                    else:
                        srcrows = [(0, 128), (128, 128), (256, 16)]
                        base = kk_blk * FH
                        srct = hrepTc[l][ch]
                        colrange = (0, chcols[ch])
                    for (r0, kk) in srcrows:
                        st = dsb.tile([128, maxcols], BF16, tag=f"ds{r0}")
                        nc.sync.dma_start(
                            out=st[:kk, 0:colrange[1] - colrange[0]],
                            in_=srct[base + r0:base + r0 + kk,
                                     colrange[0]:colrange[1]])
                        strips.append((st, kk))
                    dense_range(l, kk_blk, strips, range(t0, t1),
                                lambda i: (i - t0) * 128)

            def dumfix(l):
                for kk_blk in range(N_CORES):
                    nc.sync.dma_start(out=T[l][kk_blk * PB + L:kk_blk * PB + PB, :],
                                      in_=dum_t[:])

            # ---- edge phase
            idx_off_by_tile = []
            off = 0
            for ii in range(NT):
                idx_off_by_tile.append(off)
                off += sum(cc * 8 for (_, _, cc) in plan[ii])

            def ag_issue(lnext, ch):
                eng = nc.gpsimd if ag_engine == "pool" else nc.tensor
                cbass.BassGpSimd.collective_compute(
                    eng, "AllGather", OP.bypass,
                    replica_groups=[list(range(N_CORES))],
                    ins=[hTc[lnext][ch][:]], outs=[hrepTc[lnext][ch][:]])

            def edge_tile(l, i):
                TA = T[l][0:HALF, :]
                TB = T[l][HALF:NROWS, :]
                ioff = idx_off_by_tile[i]
                # gather all chunks up-front (plan order), track first A / first B
                gts = []
                co = ioff
                for (hf, c0, cc) in plan[i]:
                    tbl = TA if hf == "A" else TB
                    g_t = gp.tile([128, CMAX, CT], BF16, tag="g")
                    nc.gpsimd.dma_gather(out_ap=g_t[:, 0:cc, :], in_ap=tbl,
                                         idxs_ap=idx_s[:, co:co + cc * 8],
                                         num_idxs=128 * cc, num_idxs_reg=128 * cc,
                                         elem_size=CT, single_packet=False)
                    co += cc * 8
                    gts.append((hf, c0, cc, g_t))
                gA0 = next(g for (hf, c0, cc, g) in gts if hf == "A" and c0 == 0)
                gB0 = next(g for (hf, c0, cc, g) in gts if hf == "B" and c0 == 0)
                # own alpha_dst from self-loop slots (col 0 of own half)
                ad_t = sb.tile([128, NH], BF16, tag="ad")
                adb = sb.tile([128, NH], BF16, tag="adb")
                nc.vector.tensor_tensor(
                    out=ad_t[:], in0=gA0[:, 0, 272:280],
                    in1=mA_s[:, i:i + 1].broadcast_to([128, NH]), op=OP.mult)
                nc.vector.tensor_tensor(
                    out=adb[:], in0=gB0[:, 0, 272:280],
                    in1=mB_s[:, i:i + 1].broadcast_to([128, NH]), op=OP.mult)
                nc.vector.tensor_tensor(out=ad_t[:], in0=ad_t[:], in1=adb[:],
                                        op=OP.add)

                acc = sb.tile([128, 264], F32, tag="acc")
                # logits -> exp for all chunks first (hides ACT latency
                # behind the other chunks' DVE work)
                wts = []
                for (hf, c0, cc, g_t) in gts:
                    lg = sb.tile([128, CMAX, NH], BF16, tag="lg")
                    nc.vector.tensor_tensor(
                        out=lg[:, 0:cc, :], in0=g_t[:, 0:cc, 264:272],
                        in1=ad_t[:].unsqueeze(1).broadcast_to([128, cc, NH]),
                        op=OP.add)
                    nc.vector.scalar_tensor_tensor(
                        out=lg[:, 0:cc, :], in0=lg[:, 0:cc, :], scalar=0.2,
                        in1=lg[:, 0:cc, :], op0=OP.mult, op1=OP.max)
                    w_t = sb.tile([128, CMAX, NH], BF16, tag="w")
                    nc.scalar.activation(w_t[:, 0:cc, :], lg[:, 0:cc, :], AF.Exp)
                    wts.append(w_t)
                first = True
                for (hf, c0, cc, g_t), w_t in zip(gts, wts):
                    tmp = sb2.tile([128, CMAX, 264], BF16, tag="tmp")
                    gv = g_t[:, 0:cc, 0:264].rearrange("p j (c h) -> p j c h", c=33)
                    wv = w_t[:, 0:cc, :].unsqueeze(2).broadcast_to([128, cc, 33, NH])
                    tv = tmp[:, 0:cc, :].rearrange("p j (c h) -> p j c h", c=33)
                    nc.vector.tensor_tensor(out=tv, in0=gv, in1=wv, op=OP.mult)
                    nn = cc
                    while nn > 2:
                        a = nn // 2
                        nc.vector.tensor_tensor(out=tmp[:, 0:a, :],
                                                in0=tmp[:, 0:a, :],
                                                in1=tmp[:, a:2 * a, :], op=OP.add)
                        if nn % 2:
                            nc.vector.tensor_tensor(out=tmp[:, 0:1, :],
                                                    in0=tmp[:, 0:1, :],
                                                    in1=tmp[:, 2 * a:2 * a + 1, :],
                                                    op=OP.add)
                        nn = a
                    if first:
                        if nn == 2:
                            nc.vector.tensor_tensor(out=acc[:], in0=tmp[:, 0, :],
                                                    in1=tmp[:, 1, :], op=OP.add)
                        else:
                            nc.vector.tensor_copy(out=acc[:], in_=tmp[:, 0, :])
                        first = False
                    else:
                        part = sb.tile([128, 264], F32, tag="part")
                        if nn == 2:
                            nc.vector.tensor_tensor(out=part[:], in0=tmp[:, 0, :],
                                                    in1=tmp[:, 1, :], op=OP.add)
                        else:
                            nc.vector.tensor_copy(out=part[:], in_=tmp[:, 0, :])
                        nc.vector.tensor_tensor(out=acc[:], in0=acc[:], in1=part[:],
                                                op=OP.add)

                rcp = sb.tile([128, NH], F32, tag="rcp")
                nc.vector.reciprocal(rcp[:], acc[:, 256:264])
                hfull = hfp.tile([128, FH], F32, tag="hfull")
                nc.vector.tensor_tensor(
                    out=hfull[:, 0:256].rearrange("p (c h) -> p c h", c=32),
                    in0=acc[:, 0:256].rearrange("p (c h) -> p c h", c=32),
                    in1=rcp[:].unsqueeze(1).broadcast_to([128, 32, NH]),
                    op=OP.mult)
                nc.vector.tensor_tensor(out=hfull[:, 0:256], in0=hfull[:, 0:256],
                                        in1=tb[l][:], op=OP.add)
                elu_(hfull[:, 0:256], sb, [128, 256], "eluh")
                nc.scalar.copy(out=hfull[:, 256:264],
                               in_=qYs_s[:, i * NH:(i + 1) * NH])
                nc.scalar.copy(out=hfull[:, 264:272], in_=onep[:])
                if l < 2:
                    ch = chunk_of(i)
                    col0 = (i - chb[ch]) * 128
                    for (offr, kk) in ((0, 128), (128, 128), (256, 16)):
                        pt = pst.tile([128, 128], F32, tag="pt")
                        nc.tensor.transpose(out=pt[:kk, :],
                                            in_=hfull[:, offr:offr + kk],
                                            identity=ident[:])
                        st = stp.tile([128, 128], BF16, tag=f"st{offr}")
                        nc.scalar.copy(out=st[:kk, :], in_=pt[:kk, :])
                        nc.sync.dma_start(
                            out=hTc[l + 1][ch][offr:offr + kk, col0:col0 + 128],
                            in_=st[:kk, :])
                else:
                    hts = []
                    for (offr, kk) in ((0, 128), (128, 128), (256, 16)):
                        pt = pst.tile([128, 128], F32, tag="pt")
                        nc.tensor.transpose(out=pt[:kk, :],
                                            in_=hfull[:, offr:offr + kk],
                                            identity=ident[:])
                        st = stp.tile([128, 128], BF16, tag=f"st{offr}")
                        nc.scalar.copy(out=st[:kk, :], in_=pt[:kk, :])
                        hts.append((st, kk))
                    u = sb.tile([128, 528], F32, tag="u")
                    for half_i in range(2):
                        pm = ps1.tile([128, 264], F32, tag="pmlp")
                        for ci, (st, kk) in enumerate(hts):
                            nc.tensor.matmul(
                                out=pm[:], lhsT=st[:kk, :],
                                rhs=fw1t[ci][0][:kk, half_i * 264:(half_i + 1) * 264],
                                start=(ci == 0), stop=(ci == 2))
                        nc.vector.tensor_tensor(
                            out=u[:, half_i * 264:(half_i + 1) * 264],
                            in0=pm[:], in1=fb1[:, half_i * 264:(half_i + 1) * 264],
                            op=OP.add)
                    elu_(u[:], sb, [128, 528], "elu_u")
                    po = ps1.tile([128, NH], F32, tag="po")
                    for ci in range(5):
                        offc = ci * 128
                        kk = min(128, 528 - offc)
                        pt = pst.tile([128, 128], F32, tag="pt")
                        nc.tensor.transpose(out=pt[:kk, :],
                                            in_=u[:, offc:offc + kk],
                                            identity=ident[:])
                        st = stp.tile([128, 128], BF16, tag="uT")
                        nc.scalar.copy(out=st[:kk, :], in_=pt[:kk, :])
                        nc.tensor.matmul(out=po[:], lhsT=st[:kk, :],
                                         rhs=fw2t[ci][0][:kk, :],
                                         start=(ci == 0), stop=(ci == 4))
                    o_t = sb.tile([128, NH], F32, tag="o_t")
                    nc.vector.tensor_tensor(out=o_t[:], in0=po[:], in1=fb2[:],
                                            op=OP.add)
                    nc.sync.dma_start(out=out[i * 128:(i + 1) * 128, :], in_=o_t[:])

            # ---- schedule
            for ch in range(nch):
                dense_chunk(0, ch)
            dumfix(0)
            for l in range(3):
                for i in range(NT):
                    edge_tile(l, i)
                    if l < 2 and i + 1 in chb[1:]:
                        ch = chb[1:].index(i + 1)
                        ag_issue(l + 1, ch)
                        dense_chunk(l + 1, ch)
                if l < 2:
                    dumfix(l + 1)

    nc.compile()
    return nc


def run(inputs, trace=False):
    from concourse.bass_utils import run_bass_kernel_spmd
    from concourse.bass_interp import get_hw_module
    adj = np.asarray(inputs["adj"])
    n = int(np.asarray(inputs["x"]).shape[0])
    prep = preprocess(adj, n)
    in_maps = host_inputs(inputs, prep)
    nc = build_program(prep)
    nc.m = get_hw_module(nc.m)
    res = run_bass_kernel_spmd(nc, in_maps, core_ids=list(range(N_CORES)),
                               trace=trace)
    outs = [np.asarray(r["out"]) for r in res.results]
    y_slots = np.zeros((NSLOT, NH), np.float32)
    for k in range(N_CORES):
        for i in range(NT):
            slot_base = (i * N_CORES + k) * 128
            y_slots[slot_base:slot_base + 128] = outs[k][i * 128:(i + 1) * 128]
    slots = prep["slots"]
    r_real = np.flatnonzero(slots >= 0)
    y = np.zeros((n, NH), np.float32)
    y[slots[r_real]] = y_slots[r_real]
    return y, res


def kernel(**inputs) -> np.ndarray:
    y, _ = run(inputs)
    return y
